# revision 29
# baseline (speedup 1.0000x reference)
"""Trainium2 Bass kernel: multi-head attention (B=4, T=2048, D=768, H=12).

Sharding: 8 cores = 4 batches x 2 head-groups (6 heads each).
Each core computes QKV projection (its heads), attention, and a partial
output projection (contraction over its 384 of 768 w_out rows).
Host unshard: out[b] = partial[2b] + partial[2b+1] (bias folded on host).

v2: single software-pipelined schedule. The attention phase is bound by
the Scalar (ACT) engine's exp throughput (~1.1us per [128,1024] tile,
192 tiles = ~212us). All other PE work (QKV projection, V~ build,
output projection) is micro-chunked (<=4 matmuls per item) and
interleaved into the per-kb exp-wait windows of the attention loop, so
the PE's idle slack under the ACT-bound steady state absorbs it.

Per-core dataflow:
  - Q^T/K^T in transposed pair-packed [128, T] bf16 tiles; K^T per-head
    zero-padded so S^T matmuls run with a full 128-partition contraction
  - V~ natural layout with a ones column per head ([tok, 65]) so P@V also
    produces softmax denominators (row 64 of au)
  - S^T = K^T.T @ Q^T -> exp via ScalarE (fused PSUM eviction, scale=1/8,
    no max-subtraction; scores bounded ~[-2.5, 2.8]) -> au^T = V~.T @ P^T
  - unit end: au PSUM -> SBUF copy (frees the single au PSUM slot), the
    denominator row is DMA-reshaped [1,1024]->[8,128] so the reciprocal
    runs on 8 DVE lanes, DMA'd back to a [1,1024] row, broadcast to 64
    partitions with a contraction-1 matmul against an all-ones row, and
    multiplied into the out-proj lhsT layout (deferred by one unit so the
    PE never waits on the reciprocal chain)
  - out-proj from attnN^T pair tiles; b_out added on host during unshard
  - matmuls in float32r (1 cycle/row at N>=256); f32r constant fills go
    through f32 twins + DVE copies (memset cannot write f32r)

Schedule: units ordered q-major (all 6 heads of query-chunk 0, then
chunk 1). Unit 0 absorbs the V~ builds just-in-time (V6[kb] completes
right before its own PV(kb)); units 1-4 absorb the remaining projection
pairs; units 7-10 absorb the out-projection of query-chunk 0; the
out-projection of chunk 1 is the tail.

This walrus build encodes at most one sync wait per instruction; Tile
emits several. _split_multi_waits() rewrites the final module, hoisting
extra waits onto same-engine nops inserted before the instruction.
"""

import numpy as np

import concourse.bass as bass
import concourse.mybir as mybir
from concourse.tile import TileContext
from concourse.bass_utils import run_bass_kernel_spmd

# problem constants (fixed by the graded nn.Module)
B, T, D = 4, 2048, 768
H, HD = 12, 64
NCORES = 8
HL = H // 2            # heads per core (2 head-groups)
NPAIR = HL // 2        # head pairs per core

F32 = mybir.dt.float32
F32R = mybir.dt.float32r
BF16 = mybir.dt.bfloat16
F8E4 = mybir.dt.float8e4


def _split_multi_waits(nc):
    """Walrus here encodes only one sync wait per instruction. Move extra
    waits onto same-engine nops placed immediately before the instruction."""
    n = 0
    for f in nc.m.functions:
        for bb in f.blocks:
            new = []
            for inst in bb.instructions:
                si = inst.sync_info
                if si is not None and si.on_wait and len(si.on_wait) > 1:
                    extra = list(si.on_wait[:-1])
                    keep = si.on_wait[-1]
                    del si.on_wait[:]
                    si.on_wait.append(keep)
                    for w in extra:
                        nop = mybir.InstNoOp(name=f"I-wsplit-{n}", ins=[], outs=[])
                        n += 1
                        nop.engine = inst.engine
                        nop.sync_info = mybir.SyncInfo(on_wait=[w], on_update=[])
                        new.append(nop)
                new.append(inst)
            bb.instructions[:] = new
    return n


def build_nc(t=T, qc=1024, nch=512):
    """Build the SPMD per-core program. qc = attention query chunk,
    nch = matmul moving-dim chunk."""
    tokt = t // 128            # token tiles
    nqc = t // qc              # query chunks
    dk = D // 128              # contraction tiles over D
    ncc = t // nch             # projection moving chunks per M row
    nmt = 2 * HL * HD // 128   # QK projection M-tiles (6)

    nc = bass.Bass("TRN2", target_bir_lowering=False, debug=False)

    xt_d = nc.dram_tensor("xt", [D, t], F32R, kind="ExternalInput")
    wqk_d = nc.dram_tensor("wqk", [D, 2 * HL * HD], F32R, kind="ExternalInput")
    bqk_d = nc.dram_tensor("bqk", [128, nmt], F32, kind="ExternalInput")
    wv_d = nc.dram_tensor("wv", [D + 1, HL * HD], F32R, kind="ExternalInput")
    wo_d = nc.dram_tensor("wo", [HL * HD, D], BF16, kind="ExternalInput")
    out_d = nc.dram_tensor("out", [t, D], F32, kind="ExternalOutput")

    def MM(out, lhsT, rhs, start, stop, perf_mode=None):
        nc.tensor.matmul(out, lhsT, rhs, start=start, stop=stop, perf_mode=perf_mode)

    with TileContext(nc) as tc:
        lp = nc.allow_low_precision(reason="float32r matmul operand production")
        lp.__enter__()
        with tc.tile_pool(name="persist", bufs=1) as pp:
            ones_row = pp.tile([1, 128], F32R, name="ones_row")
            # fp8 Q/K in DoubleRow pair layout [dims, 2, T]: rows 0:64 head
            # 2p, 64:128 head 2p+1. QT8 slots = (q8, dq8) so one DoubleRow
            # matmul computes k8·(q8+dq8) = k8·q — Q-side fp8 error cancels.
            # KT8 duplicates k8 in both slots.
            QT8 = [pp.tile([128, 2, t], F8E4, name=f"qt8_{p}") for p in range(NPAIR)]
            KT8 = [pp.tile([128, 2, t], F8E4, name=f"kt8_{p}") for p in range(NPAIR)]
            V6 = [pp.tile([128, HL * (HD + 1)], BF16, name=f"v6_{c}") for c in range(tokt)]
            bqk_t = pp.tile([128, nmt], F32, name="bqk_t")
            AN = [pp.tile([128, t], BF16, name=f"an{p}") for p in range(NPAIR)]
            WO = [pp.tile([128, D], BF16, name=f"wop{p}") for p in range(NPAIR)]
            r_pads = [pp.tile([1, qc], F32R, name=f"r_pad{i}") for i in range(2)]
            au_sbs = [pp.tile([65, qc], F32, name=f"au_sb{i}") for i in range(2)]
            den8s = [pp.tile([8, qc // 8], F32, name=f"den8_{i}") for i in range(2)]
            rec8s = [pp.tile([8, qc // 8], F32R, name=f"rec8_{i}") for i in range(2)]
            # pair-0/1 partial sums of the tail out-projection (pass A runs
            # as fills inside the last unit; pass B adds pair 2 at the tail)
            soa = [pp.tile([128, D], BF16, name=f"soa{i}") for i in range(tokt // 2)]
            xt_t = pp.tile([128, dk, t], F32R, name="xt_t")
            wqk_t = pp.tile([128, dk, 2 * HL * HD], F32R, name="wqk_t")
            wv_t = pp.tile([128, dk, HL * HD], F32R, name="wv_t")
            wvb = pp.tile([1, HL * HD], F32R, name="wvb")

            # ---- DMA emission in priority order: the first S matmul needs
            # KT[0] chunk0 + QT[0] cols 0:1024 (xt chunks 0-1 + wqk pair0);
            # V~ tiles need wv; xt chunks 2-3 aren't consumed until kb 8+.
            nc.sync.dma_start(out=bqk_t[:], in_=bqk_d[:, :])
            psl0 = slice(0, 256)
            nc.sync.dma_start(
                out=wqk_t[:, :, psl0],
                in_=wqk_d[:, psl0].rearrange("(k r) c -> r k c", k=dk),
            )
            for ch in (0, 1):
                csl = slice(ch * nch, (ch + 1) * nch)
                nc.sync.dma_start(
                    out=xt_t[:, :, csl],
                    in_=xt_d[:, csl].rearrange("(k r) c -> r k c", k=dk),
                )
            nc.sync.dma_start(
                out=wv_t[:], in_=wv_d[0:D, :].rearrange("(k r) c -> r k c", k=dk)
            )
            nc.sync.dma_start(out=wvb[0:1, :], in_=wv_d[D : D + 1, :])
            for ch in (2, 3):
                csl = slice(ch * nch, (ch + 1) * nch)
                nc.sync.dma_start(
                    out=xt_t[:, :, csl],
                    in_=xt_d[:, csl].rearrange("(k r) c -> r k c", k=dk),
                )
            for p_ in (1, 2):
                psl = slice(p_ * 256, (p_ + 1) * 256)
                nc.sync.dma_start(
                    out=wqk_t[:, :, psl],
                    in_=wqk_d[:, psl].rearrange("(k r) c -> r k c", k=dk),
                )
            for p_ in range(NPAIR):
                nc.sync.dma_start(out=WO[p_][:], in_=wo_d[p_ * 128 : (p_ + 1) * 128, :])

            # ---- constants / padding init ----
            with tc.tile_pool(name="init", bufs=1) as ip:
                ones32 = ip.tile([1, 128], F32, name="ones32")
                nc.vector.memset(ones32[:], 1.0)
                nc.vector.tensor_copy(ones_row[:], ones32[:])
                warm = ip.tile([1, 16], F32, name="warm")
                nc.scalar.activation(
                    warm[:], ones32[0:1, 0:16], mybir.ActivationFunctionType.Exp
                )
                for c in range(tokt):
                    v3i = V6[c][:].rearrange("p (h c) -> p h c", c=HD + 1)
                    nc.vector.memset(v3i[:, :, HD : HD + 1], 1.0)

            with (
                tc.tile_pool(name="ps_s", bufs=2, space="PSUM") as s_pool,
                tc.tile_pool(name="ps_u", bufs=1, space="PSUM") as u_pool,
                tc.tile_pool(name="ps_x", bufs=2, space="PSUM") as x_pool,
                tc.tile_pool(name="sb_pt", bufs=3) as ptp,
                tc.tile_pool(name="sb_r", bufs=2) as rsp,
                tc.tile_pool(name="sb_o", bufs=3) as osp,
            ):
                # ---------- micro-item emitters ----------
                aux_state = {}

                def qk_half(p_, m, c, half):
                    """Half of one QK-projection chunk: 3 of 6 k-matmuls into
                    an aux PSUM slot; second half evicts to QT/KT."""
                    key = ("qk", p_, m, c)
                    gm = 2 * p_ + m
                    csl = slice(c * nch, (c + 1) * nch)
                    if half == 0:
                        ps = x_pool.tile([128, nch], F32, tag="x", bufs=2, name="psqk")
                        aux_state[key] = ps
                        ks = range(0, dk // 2)
                    else:
                        ps = aux_state.pop(key)
                        ks = range(dk // 2, dk)
                    for k in ks:
                        MM(
                            ps[:],
                            wqk_t[:, k, gm * 128 : (gm + 1) * 128],
                            xt_t[:, k, csl],
                            start=(k == 0),
                            stop=(k == dk - 1),
                        )
                    if half == 1:
                        if m == 0:
                            # q8 into slot 0, residual dq8 = (q - q8) into slot 1
                            nc.vector.tensor_scalar_add(
                                QT8[p_][:, 0, csl], ps[:], bqk_t[:, gm : gm + 1]
                            )
                            nc.vector.scalar_tensor_tensor(
                                QT8[p_][:, 1, csl],
                                ps[:],
                                bqk_t[:, gm : gm + 1],
                                QT8[p_][:, 0, csl],
                                op0=mybir.AluOpType.add,
                                op1=mybir.AluOpType.subtract,
                            )
                        else:
                            # k8 duplicated into both DoubleRow slots
                            nc.vector.tensor_scalar_add(
                                KT8[p_][:, 0, csl], ps[:], bqk_t[:, gm : gm + 1]
                            )
                            nc.vector.tensor_copy(KT8[p_][:, 1, csl], KT8[p_][:, 0, csl])

                def v6_half(c, half):
                    """Half of one V~ tile build: k-matmuls into aux PSUM;
                    second half adds bias (contraction-1 matmul) and scatters
                    into V6[c] with the per-head ones column."""
                    key = ("v6", c)
                    tsl = slice(c * 128, (c + 1) * 128)
                    if half == 0:
                        psv = x_pool.tile(
                            [128, HL * HD], F32, tag="x", bufs=2, name="psv"
                        )
                        aux_state[key] = psv
                        for k in range(0, dk // 2):
                            MM(psv[:], xt_t[:, k, tsl], wv_t[:, k, :], start=(k == 0), stop=False)
                    else:
                        psv = aux_state.pop(key)
                        for k in range(dk // 2, dk):
                            MM(psv[:], xt_t[:, k, tsl], wv_t[:, k, :], start=False, stop=False)
                        MM(psv[:], ones_row[0:1, 0:128], wvb[0:1, :], start=False, stop=True)
                        v3 = V6[c][:].rearrange("p (h c) -> p h c", c=HD + 1)
                        nc.vector.tensor_copy(
                            v3[:, :, 0:HD],
                            psv[:].rearrange("p (h c) -> p h c", c=HD),
                        )

                def oproj_half(c, half):
                    """Half of one out-projection token tile: 3 pair-matmuls
                    over one 384-column slice of D, evicted into a staging
                    SBUF tile; second half DMAs the tile out."""
                    key = ("op", c)
                    tsl = slice(c * 128, (c + 1) * 128)
                    nsl = slice(half * (D // 2), (half + 1) * (D // 2))
                    ps = x_pool.tile([128, D // 2], F32, tag="x", bufs=2, name="pso")
                    if half == 0:
                        so = osp.tile([128, D], F32, tag="so", bufs=3, name="so")
                        aux_state[key] = so
                    else:
                        so = aux_state.pop(key)
                    for p_ in range(NPAIR):
                        MM(
                            ps[:],
                            AN[p_][:, tsl],
                            WO[p_][:, nsl],
                            start=(p_ == 0),
                            stop=(p_ == NPAIR - 1),
                        )
                    nc.vector.tensor_copy(so[:, nsl], ps[:])
                    if half == 1:
                        nc.sync.dma_start(out=out_d[tsl, :], in_=so[:])

                def oproj_a_half(c, half):
                    """Tail out-proj pass A: pairs 0+1 only, into a persistent
                    SBUF accumulator (pair 2's AN isn't normalized yet)."""
                    tsl = slice(c * 128, (c + 1) * 128)
                    nsl = slice(half * (D // 2), (half + 1) * (D // 2))
                    ps = x_pool.tile([128, D // 2], F32, tag="x", bufs=2, name="psa")
                    for p_ in (0, 1):
                        MM(ps[:], AN[p_][:, tsl], WO[p_][:, nsl], start=(p_ == 0), stop=(p_ == 1))
                    nc.vector.tensor_copy(soa[c - tokt // 2][:, nsl], ps[:])

                def oproj_b_half(c, half, eng):
                    """Tail out-proj pass B: pair 2 matmul + add of pass A."""
                    key = ("opb", c)
                    tsl = slice(c * 128, (c + 1) * 128)
                    nsl = slice(half * (D // 2), (half + 1) * (D // 2))
                    ps = x_pool.tile([128, D // 2], F32, tag="x", bufs=2, name="psb")
                    if half == 0:
                        so = osp.tile([128, D], F32, tag="so", bufs=3, name="so")
                        aux_state[key] = so
                    else:
                        so = aux_state.pop(key)
                    MM(ps[:], AN[2][:, tsl], WO[2][:, nsl], start=True, stop=True)
                    eng.tensor_tensor(
                        so[:, nsl], ps[:], soa[c - tokt // 2][:, nsl],
                        op=mybir.AluOpType.add,
                    )
                    if half == 1:
                        nc.sync.dma_start(out=out_d[tsl, :], in_=so[:])

                def finish_unit(u):
                    """Deferred normalize: broadcast the reciprocal row to 64
                    partitions (contraction-1 matmuls), multiply into AN."""
                    up, uj, uq, uau_sb, urp = u
                    uqsl = slice(uq * qc, (uq + 1) * qc)
                    R_sb = rsp.tile([64, qc], F32, tag="rsb", bufs=2, name="R_sb")
                    for c in range(qc // nch):
                        csl = slice(c * nch, (c + 1) * nch)
                        R = x_pool.tile([64, nch], F32, tag="x", bufs=2, name="Rp")
                        MM(R[:], ones_row[0:1, 0:64], urp[0:1, csl], start=True, stop=True)
                        nc.vector.tensor_copy(R_sb[:, csl], R[:])
                    nc.vector.tensor_mul(
                        AN[up][uj * 64 : (uj + 1) * 64, uqsl], uau_sb[0:64, :], R_sb[:]
                    )

                # ---------- fill schedules ----------
                def v6_items():
                    return [
                        (lambda c=c, hf=hf: v6_half(c, hf))
                        for c in range(tokt)
                        for hf in range(2)
                    ]

                def qk_items(p_, m, cs):
                    return [
                        (lambda c=c, hf=hf: qk_half(p_, m, c, hf))
                        for c in cs
                        for hf in range(2)
                    ]

                def op_items(cs):
                    return [
                        (lambda c=c, hf=hf: oproj_half(c, hf))
                        for c in cs
                        for hf in range(2)
                    ]

                v6h = v6_items()
                fills = {i: [] for i in range(2 * HL)}
                # unit 0 absorbs the rest of KT pair0 (chunk c ready before
                # S(4c) consumes it) and all V~ builds (V6[c] ready before the
                # trailing PV(c))
                fills[0] = (
                    qk_items(0, 1, [1]) + v6h[0:4]
                    + qk_items(0, 1, [2]) + v6h[4:12]
                    + qk_items(0, 1, [3]) + v6h[12:32]
                )
                fills[1] = qk_items(1, 1, range(ncc)) + qk_items(1, 0, [0, 1])
                fills[2] = qk_items(2, 1, range(ncc))              # KT pair2
                fills[3] = qk_items(2, 0, [0, 1]) + qk_items(1, 0, [2, 3])
                fills[4] = qk_items(2, 0, [2, 3]) + qk_items(0, 0, [2, 3])
                fills[7] = op_items([0, 1])
                fills[8] = op_items([2, 3])
                fills[9] = op_items([4, 5])
                fills[10] = op_items([6, 7])
                fills[11] = [
                    (lambda c=c, hf=hf: oproj_a_half(c, hf))
                    for c in range(tokt // 2, tokt)
                    for hf in range(2)
                ]

                # ---- projection preamble: KT pair0 chunk0, QT pair0 q0-half ----
                qk_half(0, 1, 0, 0)
                qk_half(0, 1, 0, 1)
                for c in (0, 1):
                    qk_half(0, 0, c, 0)
                    qk_half(0, 0, c, 1)

                # ---- attention units, q-major ----
                units = [
                    (q, p_, j)
                    for q in range(nqc)
                    for p_ in range(NPAIR)
                    for j in range(2)
                ]
                pending = None
                unit_no = 0
                for ui, (q, p_, j) in enumerate(units):
                    h = 2 * p_ + j
                    fl = fills[ui]
                    au = u_pool.tile([65, qc], F32, tag="au", bufs=1, name="au")

                    def emit_pv(okb, opt):
                        for c in range(qc // nch):
                            MM(
                                au[:, c * nch : (c + 1) * nch],
                                V6[okb][:, h * (HD + 1) : (h + 1) * (HD + 1)],
                                opt[:, c * nch : (c + 1) * nch],
                                start=(okb == 0),
                                stop=(okb == tokt - 1),
                            )

                    # PV emission trails exp by L steps: the PE queue is
                    # in-order, so a trailing PV gives the PE ready work while
                    # exp(kb) is still running, and in unit 0 it gives the V~
                    # fills time to land before their PV consumes them.
                    L = 6 if ui == 0 else 2
                    pvq = []
                    for kb in range(tokt):
                        if kb == 10 and pending is not None:
                            finish_unit(pending)
                            pending = None
                        st = s_pool.tile([128, qc], F32, tag="st", bufs=2, name="st")
                        jb = j * 64
                        for c in range(qc // 256):
                            MM(
                                st[:, c * 256 : (c + 1) * 256],
                                KT8[p_][jb : jb + 64, :, kb * 128 : (kb + 1) * 128],
                                QT8[p_][jb : jb + 64, :, q * qc + c * 256 : q * qc + (c + 1) * 256],
                                start=True,
                                stop=True,
                                perf_mode=mybir.MatmulPerfMode.DoubleRow,
                            )
                        pt = ptp.tile([128, qc], BF16, tag="pt", bufs=8, name="pt")
                        nc.scalar.activation(
                            pt[:], st[:], mybir.ActivationFunctionType.Exp, scale=0.125
                        )
                        pvq.append((kb, pt))
                        # fills sit in the PE's exp-wait window (before PV)
                        npop = (2 if kb < 8 else 3) if ui == 0 else 1
                        for _ in range(npop):
                            if fl:
                                fl.pop(0)()
                        if len(pvq) > L:
                            emit_pv(*pvq.pop(0))
                    for okb, opt in pvq:
                        emit_pv(okb, opt)
                    # unit end: evict au, launch the reciprocal chain
                    au_sb = au_sbs[unit_no % 2]
                    nc.vector.tensor_copy(au_sb[:], au[:])
                    rp_t = r_pads[unit_no % 2]
                    den8 = den8s[unit_no % 2]
                    rec8 = rec8s[unit_no % 2]
                    nc.sync.dma_start(out=den8[:], in_=au_sb[64:65, :])
                    nc.vector.reciprocal(rec8[:], den8[:])
                    nc.sync.dma_start(out=rp_t[0:1, :], in_=rec8[:])
                    if pending is not None:
                        finish_unit(pending)
                    pending = (p_, j, q, au_sb, rp_t)
                    unit_no += 1
                if pending is not None:
                    finish_unit(pending)

                # ---- tail: pass B of the last query chunk's out-projection
                # (pair 2 only; pass A ran as fills in the last unit). The
                # adds alternate DVE/GpSimd so neither engine serializes it.
                for c in range(tokt // 2, tokt):
                    for hf in range(2):
                        oproj_b_half(c, hf, nc.vector)
        lp.__exit__(None, None, None)

    return nc


def shard_inputs(x, w_qkv, b_qkv, w_out, b_out, t=T):
    """Build the 8 per-core input maps. Core = (batch, head-group)."""
    in_maps = []
    for core in range(NCORES):
        b, g = divmod(core, 2)
        hbase = HL * g * HD          # first qk column of this group (384*g)
        # q cols then k cols, pair-interleaved: M-tile 2p = q of heads (2p,2p+1),
        # M-tile 2p+1 = k of the same heads.
        wqk = np.empty((D, 2 * HL * HD), dtype=np.float32)
        bqk = np.empty((2 * HL * HD,), dtype=np.float32)
        for p in range(NPAIR):
            qcols = slice(0 * D + hbase + p * 128, 0 * D + hbase + (p + 1) * 128)
            kcols = slice(1 * D + hbase + p * 128, 1 * D + hbase + (p + 1) * 128)
            wqk[:, (2 * p) * 128 : (2 * p + 1) * 128] = w_qkv[:, qcols]
            wqk[:, (2 * p + 1) * 128 : (2 * p + 2) * 128] = w_qkv[:, kcols]
            bqk[(2 * p) * 128 : (2 * p + 1) * 128] = b_qkv[qcols]
            bqk[(2 * p + 1) * 128 : (2 * p + 2) * 128] = b_qkv[kcols]
        nmt = 2 * HL * HD // 128
        bqk_col = np.ascontiguousarray(bqk.reshape(nmt, 128).T)  # [128, nmt]

        vcols = slice(2 * D + hbase, 2 * D + hbase + HL * HD)
        wv = np.empty((D + 1, HL * HD), dtype=np.float32)
        wv[:D] = w_qkv[:, vcols]
        wv[D] = b_qkv[vcols]

        import ml_dtypes

        wo = np.ascontiguousarray(w_out[hbase : hbase + HL * HD, :]).astype(
            ml_dtypes.bfloat16
        )

        in_maps.append(
            {
                "xt": np.ascontiguousarray(x[b, :t].T),
                "wqk": wqk,
                "bqk": bqk_col,
                "wv": wv,
                "wo": wo,
            }
        )
    return in_maps


def kernel(x, w_qkv, b_qkv, w_out, b_out):
    x = np.asarray(x, dtype=np.float32)
    w_qkv = np.asarray(w_qkv, dtype=np.float32)
    b_qkv = np.asarray(b_qkv, dtype=np.float32)
    w_out = np.asarray(w_out, dtype=np.float32)
    b_out = np.asarray(b_out, dtype=np.float32)

    nc = build_nc()
    _split_multi_waits(nc)
    in_maps = shard_inputs(x, w_qkv, b_qkv, w_out, b_out)
    res = run_bass_kernel_spmd(nc, in_maps, list(range(NCORES)))
    parts = [np.asarray(res.results[i]["out"]) for i in range(NCORES)]
    out = np.stack([parts[2 * b] + parts[2 * b + 1] for b in range(B)], axis=0)
    out += b_out[None, None, :]
    return out.astype(np.float32)


# revision 34
# speedup vs baseline: 1.1445x; 1.1445x over previous
"""Trainium2 Bass kernel: multi-head attention (B=4, T=2048, D=768, H=12).

Sharding: 8 cores = 4 batches x 2 head-groups (6 heads each).
Each core computes QKV projection (its heads), attention, and a partial
output projection (contraction over its 384 of 768 w_out rows).
Host unshard: out[b] = partial[2b] + partial[2b+1] (bias folded on host).

v2: single software-pipelined schedule. The attention phase is bound by
the Scalar (ACT) engine's exp throughput (~1.1us per [128,1024] tile,
192 tiles = ~212us). All other PE work (QKV projection, V~ build,
output projection) is micro-chunked (<=4 matmuls per item) and
interleaved into the per-kb exp-wait windows of the attention loop, so
the PE's idle slack under the ACT-bound steady state absorbs it.

Per-core dataflow:
  - Q^T/K^T in transposed pair-packed [128, T] bf16 tiles; K^T per-head
    zero-padded so S^T matmuls run with a full 128-partition contraction
  - V~ natural layout with a ones column per head ([tok, 65]) so P@V also
    produces softmax denominators (row 64 of au)
  - S^T = K^T.T @ Q^T -> exp via ScalarE (fused PSUM eviction, scale=1/8,
    no max-subtraction; scores bounded ~[-2.5, 2.8]) -> au^T = V~.T @ P^T
  - unit end: au PSUM -> SBUF copy (frees the single au PSUM slot), the
    denominator row is DMA-reshaped [1,1024]->[8,128] so the reciprocal
    runs on 8 DVE lanes, DMA'd back to a [1,1024] row, broadcast to 64
    partitions with a contraction-1 matmul against an all-ones row, and
    multiplied into the out-proj lhsT layout (deferred by one unit so the
    PE never waits on the reciprocal chain)
  - out-proj from attnN^T pair tiles; b_out added on host during unshard
  - matmuls in float32r (1 cycle/row at N>=256); f32r constant fills go
    through f32 twins + DVE copies (memset cannot write f32r)

Schedule: units ordered q-major (all 6 heads of query-chunk 0, then
chunk 1). Unit 0 absorbs the V~ builds just-in-time (V6[kb] completes
right before its own PV(kb)); units 1-4 absorb the remaining projection
pairs; units 7-10 absorb the out-projection of query-chunk 0; the
out-projection of chunk 1 is the tail.

This walrus build encodes at most one sync wait per instruction; Tile
emits several. _split_multi_waits() rewrites the final module, hoisting
extra waits onto same-engine nops inserted before the instruction.
"""

import numpy as np

import concourse.bass as bass
import concourse.mybir as mybir
from concourse.tile import TileContext
from concourse.bass_utils import run_bass_kernel_spmd

# problem constants (fixed by the graded nn.Module)
B, T, D = 4, 2048, 768
H, HD = 12, 64
NCORES = 8
HL = H // 2            # heads per core (2 head-groups)
NPAIR = HL // 2        # head pairs per core

F32 = mybir.dt.float32
F32R = mybir.dt.float32r
BF16 = mybir.dt.bfloat16


def _split_multi_waits(nc):
    """Walrus here encodes only one sync wait per instruction. Move extra
    waits onto same-engine nops placed immediately before the instruction."""
    n = 0
    for f in nc.m.functions:
        for bb in f.blocks:
            new = []
            for inst in bb.instructions:
                si = inst.sync_info
                if si is not None and si.on_wait and len(si.on_wait) > 1:
                    extra = list(si.on_wait[:-1])
                    keep = si.on_wait[-1]
                    del si.on_wait[:]
                    si.on_wait.append(keep)
                    for w in extra:
                        nop = mybir.InstNoOp(name=f"I-wsplit-{n}", ins=[], outs=[])
                        n += 1
                        nop.engine = inst.engine
                        nop.sync_info = mybir.SyncInfo(on_wait=[w], on_update=[])
                        new.append(nop)
                new.append(inst)
            bb.instructions[:] = new
    return n


def build_nc(t=T, qc=1024, nch=512):
    """Build the SPMD per-core program. qc = attention query chunk,
    nch = matmul moving-dim chunk."""
    tokt = t // 128            # token tiles
    nqc = t // qc              # query chunks
    dk = D // 128              # contraction tiles over D
    ncc = t // nch             # projection moving chunks per M row
    nmt = 2 * HL * HD // 128   # QK projection M-tiles (6)

    nc = bass.Bass("TRN2", target_bir_lowering=False, debug=False)

    xt_d = nc.dram_tensor("xt", [D, t], F32R, kind="ExternalInput")
    wqk_d = nc.dram_tensor("wqk", [D, 2 * HL * HD], F32R, kind="ExternalInput")
    bqk_d = nc.dram_tensor("bqk", [128, nmt], F32, kind="ExternalInput")
    wv_d = nc.dram_tensor("wv", [D + 1, HL * HD], F32R, kind="ExternalInput")
    wo_d = nc.dram_tensor("wo", [HL * HD, D], BF16, kind="ExternalInput")
    out_d = nc.dram_tensor("out", [t, D], F32, kind="ExternalOutput")

    def MM(out, lhsT, rhs, start, stop):
        nc.tensor.matmul(out, lhsT, rhs, start=start, stop=stop)

    with TileContext(nc) as tc:
        lp = nc.allow_low_precision(reason="float32r matmul operand production")
        lp.__enter__()
        with tc.tile_pool(name="persist", bufs=1) as pp:
            ones_row = pp.tile([1, 128], F32R, name="ones_row")
            QT = [pp.tile([128, t], BF16, name=f"qt{p}") for p in range(NPAIR)]
            KT = [pp.tile([128, t], BF16, name=f"kt{h}") for h in range(HL)]
            V6 = [pp.tile([128, HL * (HD + 1)], BF16, name=f"v6_{c}") for c in range(tokt)]
            bqk_t = pp.tile([128, nmt], F32, name="bqk_t")
            AN = [pp.tile([128, t], BF16, name=f"an{p}") for p in range(NPAIR)]
            WO = [pp.tile([128, D], BF16, name=f"wop{p}") for p in range(NPAIR)]
            r_pads = [pp.tile([1, qc], F32R, name=f"r_pad{i}") for i in range(2)]
            au_sbs = [pp.tile([65, qc], F32, name=f"au_sb{i}") for i in range(2)]
            den8s = [pp.tile([8, qc // 8], F32, name=f"den8_{i}") for i in range(2)]
            rec8s = [pp.tile([8, qc // 8], F32R, name=f"rec8_{i}") for i in range(2)]
            # pair-0/1 partial sums of the tail out-projection (pass A runs
            # as fills inside the last unit; pass B adds pair 2 at the tail)
            soa = [pp.tile([128, D], BF16, name=f"soa{i}") for i in range(tokt // 2)]
            xt_t = pp.tile([128, dk, t], F32R, name="xt_t")
            wqk_t = pp.tile([128, dk, 2 * HL * HD], F32R, name="wqk_t")
            wv_t = pp.tile([128, dk, HL * HD], F32R, name="wv_t")
            wvb = pp.tile([1, HL * HD], F32R, name="wvb")

            # ---- DMA emission in priority order: the first S matmul needs
            # KT[0] chunk0 + QT[0] cols 0:1024 (xt chunks 0-1 + wqk pair0);
            # V~ tiles need wv; xt chunks 2-3 aren't consumed until kb 8+.
            nc.sync.dma_start(out=bqk_t[:], in_=bqk_d[:, :])
            psl0 = slice(0, 256)
            nc.sync.dma_start(
                out=wqk_t[:, :, psl0],
                in_=wqk_d[:, psl0].rearrange("(k r) c -> r k c", k=dk),
            )
            for ch in (0, 1):
                csl = slice(ch * nch, (ch + 1) * nch)
                nc.sync.dma_start(
                    out=xt_t[:, :, csl],
                    in_=xt_d[:, csl].rearrange("(k r) c -> r k c", k=dk),
                )
            nc.sync.dma_start(
                out=wv_t[:], in_=wv_d[0:D, :].rearrange("(k r) c -> r k c", k=dk)
            )
            nc.sync.dma_start(out=wvb[0:1, :], in_=wv_d[D : D + 1, :])
            for ch in (2, 3):
                csl = slice(ch * nch, (ch + 1) * nch)
                nc.sync.dma_start(
                    out=xt_t[:, :, csl],
                    in_=xt_d[:, csl].rearrange("(k r) c -> r k c", k=dk),
                )
            for p_ in (1, 2):
                psl = slice(p_ * 256, (p_ + 1) * 256)
                nc.sync.dma_start(
                    out=wqk_t[:, :, psl],
                    in_=wqk_d[:, psl].rearrange("(k r) c -> r k c", k=dk),
                )
            for p_ in range(NPAIR):
                nc.sync.dma_start(out=WO[p_][:], in_=wo_d[p_ * 128 : (p_ + 1) * 128, :])

            # ---- constants / padding init ----
            with tc.tile_pool(name="init", bufs=1) as ip:
                ones32 = ip.tile([1, 128], F32, name="ones32")
                nc.vector.memset(ones32[:], 1.0)
                nc.vector.tensor_copy(ones_row[:], ones32[:])
                warm = ip.tile([1, 16], F32, name="warm")
                nc.scalar.activation(
                    warm[:], ones32[0:1, 0:16], mybir.ActivationFunctionType.Exp
                )

                for h in range(HL):
                    if h % 2 == 0:
                        nc.vector.memset(KT[h][64:128, :], 0.0)
                    else:
                        nc.vector.memset(KT[h][0:64, :], 0.0)
                for c in range(tokt):
                    v3i = V6[c][:].rearrange("p (h c) -> p h c", c=HD + 1)
                    nc.vector.memset(v3i[:, :, HD : HD + 1], 1.0)

            with (
                tc.tile_pool(name="ps_s", bufs=2, space="PSUM") as s_pool,
                tc.tile_pool(name="ps_u", bufs=1, space="PSUM") as u_pool,
                tc.tile_pool(name="ps_x", bufs=2, space="PSUM") as x_pool,
                tc.tile_pool(name="sb_pt", bufs=3) as ptp,
                tc.tile_pool(name="sb_r", bufs=2) as rsp,
                tc.tile_pool(name="sb_o", bufs=3) as osp,
            ):
                # ---------- micro-item emitters ----------
                aux_state = {}

                def qk_half(p_, m, c, half):
                    """Half of one QK-projection chunk: 3 of 6 k-matmuls into
                    an aux PSUM slot; second half evicts to QT/KT."""
                    key = ("qk", p_, m, c)
                    gm = 2 * p_ + m
                    csl = slice(c * nch, (c + 1) * nch)
                    if half == 0:
                        ps = x_pool.tile([128, nch], F32, tag="x", bufs=2, name="psqk")
                        aux_state[key] = ps
                        ks = range(0, dk // 2)
                    else:
                        ps = aux_state.pop(key)
                        ks = range(dk // 2, dk)
                    for k in ks:
                        MM(
                            ps[:],
                            wqk_t[:, k, gm * 128 : (gm + 1) * 128],
                            xt_t[:, k, csl],
                            start=(k == 0),
                            stop=(k == dk - 1),
                        )
                    if half == 1:
                        if m == 0:
                            nc.vector.tensor_scalar_add(
                                QT[p_][:, csl], ps[:], bqk_t[:, gm : gm + 1]
                            )
                        else:
                            h0, h1 = 2 * p_, 2 * p_ + 1
                            nc.vector.tensor_scalar_add(
                                KT[h0][0:64, csl], ps[0:64, :], bqk_t[0:64, gm : gm + 1]
                            )
                            nc.vector.tensor_scalar_add(
                                KT[h1][64:128, csl], ps[64:128, :], bqk_t[64:128, gm : gm + 1]
                            )

                def v6_half(c, half):
                    """Half of one V~ tile build: k-matmuls into aux PSUM;
                    second half adds bias (contraction-1 matmul) and scatters
                    into V6[c] with the per-head ones column."""
                    key = ("v6", c)
                    tsl = slice(c * 128, (c + 1) * 128)
                    if half == 0:
                        psv = x_pool.tile(
                            [128, HL * HD], F32, tag="x", bufs=2, name="psv"
                        )
                        aux_state[key] = psv
                        for k in range(0, dk // 2):
                            MM(psv[:], xt_t[:, k, tsl], wv_t[:, k, :], start=(k == 0), stop=False)
                    else:
                        psv = aux_state.pop(key)
                        for k in range(dk // 2, dk):
                            MM(psv[:], xt_t[:, k, tsl], wv_t[:, k, :], start=False, stop=False)
                        MM(psv[:], ones_row[0:1, 0:128], wvb[0:1, :], start=False, stop=True)
                        v3 = V6[c][:].rearrange("p (h c) -> p h c", c=HD + 1)
                        nc.vector.tensor_copy(
                            v3[:, :, 0:HD],
                            psv[:].rearrange("p (h c) -> p h c", c=HD),
                        )

                def oproj_half(c, half):
                    """Half of one out-projection token tile: 3 pair-matmuls
                    over one 384-column slice of D, evicted into a staging
                    SBUF tile; second half DMAs the tile out."""
                    key = ("op", c)
                    tsl = slice(c * 128, (c + 1) * 128)
                    nsl = slice(half * (D // 2), (half + 1) * (D // 2))
                    ps = x_pool.tile([128, D // 2], F32, tag="x", bufs=2, name="pso")
                    if half == 0:
                        so = osp.tile([128, D], F32, tag="so", bufs=3, name="so")
                        aux_state[key] = so
                    else:
                        so = aux_state.pop(key)
                    for p_ in range(NPAIR):
                        MM(
                            ps[:],
                            AN[p_][:, tsl],
                            WO[p_][:, nsl],
                            start=(p_ == 0),
                            stop=(p_ == NPAIR - 1),
                        )
                    nc.vector.tensor_copy(so[:, nsl], ps[:])
                    if half == 1:
                        nc.sync.dma_start(out=out_d[tsl, :], in_=so[:])

                def oproj_a_half(c, half):
                    """Tail out-proj pass A: pairs 0+1 only, into a persistent
                    SBUF accumulator (pair 2's AN isn't normalized yet)."""
                    tsl = slice(c * 128, (c + 1) * 128)
                    nsl = slice(half * (D // 2), (half + 1) * (D // 2))
                    ps = x_pool.tile([128, D // 2], F32, tag="x", bufs=2, name="psa")
                    for p_ in (0, 1):
                        MM(ps[:], AN[p_][:, tsl], WO[p_][:, nsl], start=(p_ == 0), stop=(p_ == 1))
                    nc.vector.tensor_copy(soa[c - tokt // 2][:, nsl], ps[:])

                def oproj_b_half(c, half, eng):
                    """Tail out-proj pass B: pair 2 matmul + add of pass A."""
                    key = ("opb", c)
                    tsl = slice(c * 128, (c + 1) * 128)
                    nsl = slice(half * (D // 2), (half + 1) * (D // 2))
                    ps = x_pool.tile([128, D // 2], F32, tag="x", bufs=2, name="psb")
                    if half == 0:
                        so = osp.tile([128, D], F32, tag="so", bufs=3, name="so")
                        aux_state[key] = so
                    else:
                        so = aux_state.pop(key)
                    MM(ps[:], AN[2][:, tsl], WO[2][:, nsl], start=True, stop=True)
                    eng.tensor_tensor(
                        so[:, nsl], ps[:], soa[c - tokt // 2][:, nsl],
                        op=mybir.AluOpType.add,
                    )
                    if half == 1:
                        nc.sync.dma_start(out=out_d[tsl, :], in_=so[:])

                def finish_unit(u):
                    """Deferred normalize: broadcast the reciprocal row to 64
                    partitions (contraction-1 matmuls), multiply into AN."""
                    up, uj, uq, uau_sb, urp = u
                    uqsl = slice(uq * qc, (uq + 1) * qc)
                    R_sb = rsp.tile([64, qc], F32, tag="rsb", bufs=2, name="R_sb")
                    for c in range(qc // nch):
                        csl = slice(c * nch, (c + 1) * nch)
                        R = x_pool.tile([64, nch], F32, tag="x", bufs=2, name="Rp")
                        MM(R[:], ones_row[0:1, 0:64], urp[0:1, csl], start=True, stop=True)
                        nc.vector.tensor_copy(R_sb[:, csl], R[:])
                    nc.vector.tensor_mul(
                        AN[up][uj * 64 : (uj + 1) * 64, uqsl], uau_sb[0:64, :], R_sb[:]
                    )

                # ---------- fill schedules ----------
                def v6_items():
                    return [
                        (lambda c=c, hf=hf: v6_half(c, hf))
                        for c in range(tokt)
                        for hf in range(2)
                    ]

                def qk_items(p_, m, cs):
                    return [
                        (lambda c=c, hf=hf: qk_half(p_, m, c, hf))
                        for c in cs
                        for hf in range(2)
                    ]

                def op_items(cs):
                    return [
                        (lambda c=c, hf=hf: oproj_half(c, hf))
                        for c in cs
                        for hf in range(2)
                    ]

                v6h = v6_items()
                fills = {i: [] for i in range(2 * HL)}
                # unit 0 absorbs the rest of KT pair0 (chunk c ready before
                # S(4c) consumes it) and all V~ builds (V6[c] ready before the
                # trailing PV(c))
                fills[0] = (
                    qk_items(0, 1, [1]) + v6h[0:4]
                    + qk_items(0, 1, [2]) + v6h[4:12]
                    + qk_items(0, 1, [3]) + v6h[12:32]
                )
                fills[1] = qk_items(1, 1, range(ncc)) + qk_items(1, 0, [0, 1])
                fills[2] = qk_items(2, 1, range(ncc))              # KT pair2
                fills[3] = qk_items(2, 0, [0, 1]) + qk_items(1, 0, [2, 3])
                fills[4] = qk_items(2, 0, [2, 3]) + qk_items(0, 0, [2, 3])
                fills[7] = op_items([0, 1])
                fills[8] = op_items([2, 3])
                fills[9] = op_items([4, 5])
                fills[10] = op_items([6, 7])
                fills[11] = [
                    (lambda c=c, hf=hf: oproj_a_half(c, hf))
                    for c in range(tokt // 2, tokt)
                    for hf in range(2)
                ]

                # ---- projection preamble: KT pair0 chunk0, QT pair0 q0-half ----
                qk_half(0, 1, 0, 0)
                qk_half(0, 1, 0, 1)
                for c in (0, 1):
                    qk_half(0, 0, c, 0)
                    qk_half(0, 0, c, 1)

                # ---- attention units, q-major ----
                units = [
                    (q, p_, j)
                    for q in range(nqc)
                    for p_ in range(NPAIR)
                    for j in range(2)
                ]
                pending = None
                unit_no = 0
                for ui, (q, p_, j) in enumerate(units):
                    h = 2 * p_ + j
                    fl = fills[ui]
                    au = u_pool.tile([65, qc], F32, tag="au", bufs=1, name="au")

                    def emit_pv(okb, opt):
                        for c in range(qc // nch):
                            MM(
                                au[:, c * nch : (c + 1) * nch],
                                V6[okb][:, h * (HD + 1) : (h + 1) * (HD + 1)],
                                opt[:, c * nch : (c + 1) * nch],
                                start=(okb == 0),
                                stop=(okb == tokt - 1),
                            )

                    # PV emission trails exp by L steps: the PE queue is
                    # in-order, so a trailing PV gives the PE ready work while
                    # exp(kb) is still running, and in unit 0 it gives the V~
                    # fills time to land before their PV consumes them.
                    L = 6 if ui == 0 else 2
                    pvq = []
                    for kb in range(tokt):
                        if kb == 10 and pending is not None:
                            finish_unit(pending)
                            pending = None
                        st = s_pool.tile([128, qc], F32, tag="st", bufs=2, name="st")
                        for c in range(qc // nch):
                            MM(
                                st[:, c * nch : (c + 1) * nch],
                                KT[h][:, kb * 128 : (kb + 1) * 128],
                                QT[p_][:, q * qc + c * nch : q * qc + (c + 1) * nch],
                                start=True,
                                stop=True,
                            )
                        pt = ptp.tile([128, qc], BF16, tag="pt", bufs=8, name="pt")
                        nc.scalar.activation(
                            pt[:], st[:], mybir.ActivationFunctionType.Exp, scale=0.125
                        )
                        pvq.append((kb, pt))
                        # fills sit in the PE's exp-wait window (before PV);
                        # pace them so the per-kb PE surplus stays under the
                        # ACT slack (one ~0.6us item per 2 kb when possible)
                        if ui == 0:
                            npop = 2 if kb < 8 else 3
                        elif len(fl) > 16 - kb:
                            npop = 1
                        elif fl and kb % 2 == 1:
                            npop = 1
                        else:
                            npop = 0
                        for _ in range(npop):
                            if fl:
                                fl.pop(0)()
                        if len(pvq) > L:
                            emit_pv(*pvq.pop(0))
                    while fl:
                        fl.pop(0)()
                    for okb, opt in pvq:
                        emit_pv(okb, opt)
                    # unit end: evict au, launch the reciprocal chain
                    au_sb = au_sbs[unit_no % 2]
                    nc.vector.tensor_copy(au_sb[:], au[:])
                    rp_t = r_pads[unit_no % 2]
                    den8 = den8s[unit_no % 2]
                    rec8 = rec8s[unit_no % 2]
                    nc.sync.dma_start(out=den8[:], in_=au_sb[64:65, :])
                    nc.vector.reciprocal(rec8[:], den8[:])
                    nc.sync.dma_start(out=rp_t[0:1, :], in_=rec8[:])
                    if pending is not None:
                        finish_unit(pending)
                    pending = (p_, j, q, au_sb, rp_t)
                    unit_no += 1
                if pending is not None:
                    finish_unit(pending)

                # ---- tail: pass B of the last query chunk's out-projection
                # (pair 2 only; pass A ran as fills in the last unit). The
                # adds alternate DVE/GpSimd so neither engine serializes it.
                for c in range(tokt // 2, tokt):
                    for hf in range(2):
                        oproj_b_half(c, hf, nc.vector)
        lp.__exit__(None, None, None)

    return nc


def shard_inputs(x, w_qkv, b_qkv, w_out, b_out, t=T):
    """Build the 8 per-core input maps. Core = (batch, head-group)."""
    in_maps = []
    for core in range(NCORES):
        b, g = divmod(core, 2)
        hbase = HL * g * HD          # first qk column of this group (384*g)
        # q cols then k cols, pair-interleaved: M-tile 2p = q of heads (2p,2p+1),
        # M-tile 2p+1 = k of the same heads.
        wqk = np.empty((D, 2 * HL * HD), dtype=np.float32)
        bqk = np.empty((2 * HL * HD,), dtype=np.float32)
        for p in range(NPAIR):
            qcols = slice(0 * D + hbase + p * 128, 0 * D + hbase + (p + 1) * 128)
            kcols = slice(1 * D + hbase + p * 128, 1 * D + hbase + (p + 1) * 128)
            wqk[:, (2 * p) * 128 : (2 * p + 1) * 128] = w_qkv[:, qcols]
            wqk[:, (2 * p + 1) * 128 : (2 * p + 2) * 128] = w_qkv[:, kcols]
            bqk[(2 * p) * 128 : (2 * p + 1) * 128] = b_qkv[qcols]
            bqk[(2 * p + 1) * 128 : (2 * p + 2) * 128] = b_qkv[kcols]
        nmt = 2 * HL * HD // 128
        bqk_col = np.ascontiguousarray(bqk.reshape(nmt, 128).T)  # [128, nmt]

        vcols = slice(2 * D + hbase, 2 * D + hbase + HL * HD)
        wv = np.empty((D + 1, HL * HD), dtype=np.float32)
        wv[:D] = w_qkv[:, vcols]
        wv[D] = b_qkv[vcols]

        import ml_dtypes

        wo = np.ascontiguousarray(w_out[hbase : hbase + HL * HD, :]).astype(
            ml_dtypes.bfloat16
        )

        in_maps.append(
            {
                "xt": np.ascontiguousarray(x[b, :t].T),
                "wqk": wqk,
                "bqk": bqk_col,
                "wv": wv,
                "wo": wo,
            }
        )
    return in_maps


def kernel(x, w_qkv, b_qkv, w_out, b_out):
    x = np.asarray(x, dtype=np.float32)
    w_qkv = np.asarray(w_qkv, dtype=np.float32)
    b_qkv = np.asarray(b_qkv, dtype=np.float32)
    w_out = np.asarray(w_out, dtype=np.float32)
    b_out = np.asarray(b_out, dtype=np.float32)

    nc = build_nc()
    _split_multi_waits(nc)
    in_maps = shard_inputs(x, w_qkv, b_qkv, w_out, b_out)
    res = run_bass_kernel_spmd(nc, in_maps, list(range(NCORES)))
    parts = [np.asarray(res.results[i]["out"]) for i in range(NCORES)]
    out = np.stack([parts[2 * b] + parts[2 * b + 1] for b in range(B)], axis=0)
    out += b_out[None, None, :]
    return out.astype(np.float32)


# revision 43
# speedup vs baseline: 1.1690x; 1.0214x over previous
"""Trainium2 Bass kernel: multi-head attention (B=4, T=2048, D=768, H=12).

Sharding: 8 cores = 4 batches x 2 head-groups (6 heads each).
Each core computes QKV projection (its heads), attention, and a partial
output projection (contraction over its 384 of 768 w_out rows).
Host unshard: out[b] = partial[2b] + partial[2b+1] (bias folded on host).

v2: single software-pipelined schedule. The attention phase is bound by
the Scalar (ACT) engine's exp throughput (~1.1us per [128,1024] tile,
192 tiles = ~212us). All other PE work (QKV projection, V~ build,
output projection) is micro-chunked (<=4 matmuls per item) and
interleaved into the per-kb exp-wait windows of the attention loop, so
the PE's idle slack under the ACT-bound steady state absorbs it.

Per-core dataflow:
  - Q^T/K^T in transposed pair-packed [128, T] bf16 tiles; K^T per-head
    zero-padded so S^T matmuls run with a full 128-partition contraction
  - V~ natural layout with a ones column per head ([tok, 65]) so P@V also
    produces softmax denominators (row 64 of au)
  - S^T = K^T.T @ Q^T -> exp via ScalarE (fused PSUM eviction, scale=1/8,
    no max-subtraction; scores bounded ~[-2.5, 2.8]) -> au^T = V~.T @ P^T
  - unit end: au PSUM -> SBUF copy (frees the single au PSUM slot), the
    denominator row is DMA-reshaped [1,1024]->[8,128] so the reciprocal
    runs on 8 DVE lanes, DMA'd back to a [1,1024] row, broadcast to 64
    partitions with a contraction-1 matmul against an all-ones row, and
    multiplied into the out-proj lhsT layout (deferred by one unit so the
    PE never waits on the reciprocal chain)
  - out-proj from attnN^T pair tiles; b_out added on host during unshard
  - matmuls in float32r (1 cycle/row at N>=256); f32r constant fills go
    through f32 twins + DVE copies (memset cannot write f32r)

Schedule: units ordered q-major (all 6 heads of query-chunk 0, then
chunk 1). Unit 0 absorbs the V~ builds just-in-time (V6[kb] completes
right before its own PV(kb)); units 1-4 absorb the remaining projection
pairs; units 7-10 absorb the out-projection of query-chunk 0; the
out-projection of chunk 1 is the tail.

This walrus build encodes at most one sync wait per instruction; Tile
emits several. _split_multi_waits() rewrites the final module, hoisting
extra waits onto same-engine nops inserted before the instruction.
"""

import numpy as np

import concourse.bass as bass
import concourse.mybir as mybir
from concourse.tile import TileContext
from concourse.bass_utils import run_bass_kernel_spmd

# problem constants (fixed by the graded nn.Module)
B, T, D = 4, 2048, 768
H, HD = 12, 64
NCORES = 8
HL = H // 2            # heads per core (2 head-groups)
NPAIR = HL // 2        # head pairs per core

F32 = mybir.dt.float32
F32R = mybir.dt.float32r
BF16 = mybir.dt.bfloat16


def _split_multi_waits(nc):
    """Walrus here encodes only one sync wait per instruction. Move extra
    waits onto same-engine nops placed immediately before the instruction."""
    n = 0
    for f in nc.m.functions:
        for bb in f.blocks:
            new = []
            for inst in bb.instructions:
                si = inst.sync_info
                if si is not None and si.on_wait and len(si.on_wait) > 1:
                    extra = list(si.on_wait[:-1])
                    keep = si.on_wait[-1]
                    del si.on_wait[:]
                    si.on_wait.append(keep)
                    for w in extra:
                        nop = mybir.InstNoOp(name=f"I-wsplit-{n}", ins=[], outs=[])
                        n += 1
                        nop.engine = inst.engine
                        nop.sync_info = mybir.SyncInfo(on_wait=[w], on_update=[])
                        new.append(nop)
                new.append(inst)
            bb.instructions[:] = new
    return n


def build_nc(t=T, qc=1024, nch=512):
    """Build the SPMD per-core program. qc = attention query chunk,
    nch = matmul moving-dim chunk."""
    tokt = t // 128            # token tiles
    nqc = t // qc              # query chunks
    dk = D // 128              # contraction tiles over D
    ncc = t // nch             # projection moving chunks per M row
    nmt = 2 * HL * HD // 128   # QK projection M-tiles (6)

    nc = bass.Bass("TRN2", target_bir_lowering=False, debug=False)

    xt_d = nc.dram_tensor("xt", [D, t], BF16, kind="ExternalInput")
    wqk_d = nc.dram_tensor("wqk", [D, 2 * HL * HD], BF16, kind="ExternalInput")
    bqk_d = nc.dram_tensor("bqk", [128, nmt], F32, kind="ExternalInput")
    wv_d = nc.dram_tensor("wv", [D + 1, HL * HD], BF16, kind="ExternalInput")
    wo_d = nc.dram_tensor("wo", [HL * HD, D], BF16, kind="ExternalInput")
    out_d = nc.dram_tensor("out", [t, D], F32, kind="ExternalOutput")

    def MM(out, lhsT, rhs, start, stop):
        nc.tensor.matmul(out, lhsT, rhs, start=start, stop=stop)

    with TileContext(nc) as tc:
        lp = nc.allow_low_precision(reason="float32r matmul operand production")
        lp.__enter__()
        with tc.tile_pool(name="persist", bufs=1) as pp:
            ones_row = pp.tile([1, 128], F32R, name="ones_row")
            ones_bf = pp.tile([1, 128], BF16, name="ones_bf")
            QT = [pp.tile([128, t], BF16, name=f"qt{p}") for p in range(NPAIR)]
            KT = [pp.tile([128, t], BF16, name=f"kt{h}") for h in range(HL)]
            V6 = [pp.tile([128, HL * (HD + 1)], BF16, name=f"v6_{c}") for c in range(tokt)]
            bqk_t = pp.tile([128, nmt], F32, name="bqk_t")
            AN = [pp.tile([128, t], BF16, name=f"an{p}") for p in range(NPAIR)]
            WO = [pp.tile([128, D], BF16, name=f"wop{p}") for p in range(NPAIR)]
            r_pads = [pp.tile([1, qc], F32R, name=f"r_pad{i}") for i in range(2)]
            au_sbs = [pp.tile([65, qc], F32, name=f"au_sb{i}") for i in range(2)]
            den8s = [pp.tile([8, qc // 8], F32, name=f"den8_{i}") for i in range(2)]
            rec8s = [pp.tile([8, qc // 8], F32R, name=f"rec8_{i}") for i in range(2)]
            # pair-0/1 partial sums of the tail out-projection (pass A runs
            # as fills inside the last unit; pass B adds pair 2 at the tail)
            soa = [pp.tile([128, D], BF16, name=f"soa{i}") for i in range(tokt // 2)]
            xt_t = pp.tile([128, dk, t], BF16, name="xt_t")
            wqk_t = pp.tile([128, dk, 2 * HL * HD], BF16, name="wqk_t")
            wv_t = pp.tile([128, dk, HL * HD], BF16, name="wv_t")
            wvb = pp.tile([1, HL * HD], BF16, name="wvb")

            # ---- DMA emission in priority order: the first S matmul needs
            # KT[0] chunk0 + QT[0] cols 0:1024 (xt chunks 0-1 + wqk pair0);
            # V~ tiles need wv; xt chunks 2-3 aren't consumed until kb 8+.
            nc.sync.dma_start(out=bqk_t[:], in_=bqk_d[:, :])

            def dma_wqk(psl):
                nc.sync.dma_start(
                    out=wqk_t[:, :, psl],
                    in_=wqk_d[:, psl].rearrange("(k r) c -> r k c", k=dk),
                )

            def dma_xt(ch):
                csl = slice(ch * nch, (ch + 1) * nch)
                nc.sync.dma_start(
                    out=xt_t[:, :, csl],
                    in_=xt_d[:, csl].rearrange("(k r) c -> r k c", k=dk),
                )

            # KT pair0 (m-tile 1) feeds the very first matmuls: its weight
            # slice goes first, the QT (m-tile 0) slice after xt chunk 0
            dma_wqk(slice(128, 256))
            dma_xt(0)
            dma_wqk(slice(0, 128))
            dma_xt(1)
            nc.sync.dma_start(
                out=wv_t[:], in_=wv_d[0:D, :].rearrange("(k r) c -> r k c", k=dk)
            )
            nc.sync.dma_start(out=wvb[0:1, :], in_=wv_d[D : D + 1, :])
            dma_xt(2)
            dma_xt(3)
            for p_ in (1, 2):
                dma_wqk(slice(p_ * 256, (p_ + 1) * 256))
            for p_ in range(NPAIR):
                nc.sync.dma_start(out=WO[p_][:], in_=wo_d[p_ * 128 : (p_ + 1) * 128, :])

            # ---- constants / padding init ----
            with tc.tile_pool(name="init", bufs=1) as ip:
                ones32 = ip.tile([1, 128], F32, name="ones32")
                nc.vector.memset(ones32[:], 1.0)
                nc.vector.tensor_copy(ones_row[:], ones32[:])
                nc.vector.memset(ones_bf[:], 1.0)
                warm = ip.tile([1, 16], F32, name="warm")
                nc.scalar.activation(
                    warm[:], ones32[0:1, 0:16], mybir.ActivationFunctionType.Exp
                )

                for h in range(HL):
                    if h % 2 == 0:
                        nc.vector.memset(KT[h][64:128, :], 0.0)
                    else:
                        nc.vector.memset(KT[h][0:64, :], 0.0)
                for c in range(tokt):
                    v3i = V6[c][:].rearrange("p (h c) -> p h c", c=HD + 1)
                    nc.vector.memset(v3i[:, :, HD : HD + 1], 1.0)

            with (
                tc.tile_pool(name="ps_s", bufs=2, space="PSUM") as s_pool,
                tc.tile_pool(name="ps_u", bufs=1, space="PSUM") as u_pool,
                tc.tile_pool(name="ps_x", bufs=2, space="PSUM") as x_pool,
                tc.tile_pool(name="sb_pt", bufs=3) as ptp,
                tc.tile_pool(name="sb_r", bufs=2) as rsp,
                tc.tile_pool(name="sb_o", bufs=3) as osp,
            ):
                # ---------- micro-item emitters ----------
                aux_state = {}

                def qk_half(p_, m, c, half):
                    """Half of one QK-projection chunk: 3 of 6 k-matmuls into
                    an aux PSUM slot; second half evicts to QT/KT."""
                    key = ("qk", p_, m, c)
                    gm = 2 * p_ + m
                    csl = slice(c * nch, (c + 1) * nch)
                    if half == 0:
                        ps = x_pool.tile([128, nch], F32, tag="x", bufs=2, name="psqk")
                        aux_state[key] = ps
                        ks = range(0, dk // 2)
                    else:
                        ps = aux_state.pop(key)
                        ks = range(dk // 2, dk)
                    for k in ks:
                        MM(
                            ps[:],
                            wqk_t[:, k, gm * 128 : (gm + 1) * 128],
                            xt_t[:, k, csl],
                            start=(k == 0),
                            stop=(k == dk - 1),
                        )
                    if half == 1:
                        if m == 0:
                            nc.vector.tensor_scalar_add(
                                QT[p_][:, csl], ps[:], bqk_t[:, gm : gm + 1]
                            )
                        else:
                            h0, h1 = 2 * p_, 2 * p_ + 1
                            nc.vector.tensor_scalar_add(
                                KT[h0][0:64, csl], ps[0:64, :], bqk_t[0:64, gm : gm + 1]
                            )
                            nc.vector.tensor_scalar_add(
                                KT[h1][64:128, csl], ps[64:128, :], bqk_t[64:128, gm : gm + 1]
                            )

                def v6_half(c, half):
                    """Half of one V~ tile build: k-matmuls into aux PSUM;
                    second half adds bias (contraction-1 matmul) and scatters
                    into V6[c] with the per-head ones column."""
                    key = ("v6", c)
                    tsl = slice(c * 128, (c + 1) * 128)
                    if half == 0:
                        psv = x_pool.tile(
                            [128, HL * HD], F32, tag="x", bufs=2, name="psv"
                        )
                        aux_state[key] = psv
                        for k in range(0, dk // 2):
                            MM(psv[:], xt_t[:, k, tsl], wv_t[:, k, :], start=(k == 0), stop=False)
                    else:
                        psv = aux_state.pop(key)
                        for k in range(dk // 2, dk):
                            MM(psv[:], xt_t[:, k, tsl], wv_t[:, k, :], start=False, stop=False)
                        MM(psv[:], ones_bf[0:1, 0:128], wvb[0:1, :], start=False, stop=True)
                        v3 = V6[c][:].rearrange("p (h c) -> p h c", c=HD + 1)
                        nc.vector.tensor_copy(
                            v3[:, :, 0:HD],
                            psv[:].rearrange("p (h c) -> p h c", c=HD),
                        )

                def oproj_half(c, half):
                    """Half of one out-projection token tile: 3 pair-matmuls
                    over one 384-column slice of D, evicted into a staging
                    SBUF tile; second half DMAs the tile out."""
                    key = ("op", c)
                    tsl = slice(c * 128, (c + 1) * 128)
                    nsl = slice(half * (D // 2), (half + 1) * (D // 2))
                    ps = x_pool.tile([128, D // 2], F32, tag="x", bufs=2, name="pso")
                    if half == 0:
                        so = osp.tile([128, D], F32, tag="so", bufs=3, name="so")
                        aux_state[key] = so
                    else:
                        so = aux_state.pop(key)
                    for p_ in range(NPAIR):
                        MM(
                            ps[:],
                            AN[p_][:, tsl],
                            WO[p_][:, nsl],
                            start=(p_ == 0),
                            stop=(p_ == NPAIR - 1),
                        )
                    nc.vector.tensor_copy(so[:, nsl], ps[:])
                    if half == 1:
                        nc.sync.dma_start(out=out_d[tsl, :], in_=so[:])

                def oproj_a_half(c, half):
                    """Tail out-proj pass A: pairs 0+1 only, into a persistent
                    SBUF accumulator (pair 2's AN isn't normalized yet)."""
                    tsl = slice(c * 128, (c + 1) * 128)
                    nsl = slice(half * (D // 2), (half + 1) * (D // 2))
                    ps = x_pool.tile([128, D // 2], F32, tag="x", bufs=2, name="psa")
                    for p_ in (0, 1):
                        MM(ps[:], AN[p_][:, tsl], WO[p_][:, nsl], start=(p_ == 0), stop=(p_ == 1))
                    nc.vector.tensor_copy(soa[c - tokt // 2][:, nsl], ps[:])

                def oproj_b_half(c, half, eng):
                    """Tail out-proj pass B: pair 2 matmul + add of pass A."""
                    key = ("opb", c)
                    tsl = slice(c * 128, (c + 1) * 128)
                    nsl = slice(half * (D // 2), (half + 1) * (D // 2))
                    ps = x_pool.tile([128, D // 2], F32, tag="x", bufs=2, name="psb")
                    if half == 0:
                        so = osp.tile([128, D], F32, tag="so", bufs=3, name="so")
                        aux_state[key] = so
                    else:
                        so = aux_state.pop(key)
                    MM(ps[:], AN[2][:, tsl], WO[2][:, nsl], start=True, stop=True)
                    eng.tensor_tensor(
                        so[:, nsl], ps[:], soa[c - tokt // 2][:, nsl],
                        op=mybir.AluOpType.add,
                    )
                    if half == 1:
                        nc.sync.dma_start(out=out_d[tsl, :], in_=so[:])

                def finish_unit(u):
                    """Deferred normalize: broadcast the reciprocal row to 64
                    partitions (contraction-1 matmuls), multiply into AN."""
                    up, uj, uq, uau_sb, urp = u
                    uqsl = slice(uq * qc, (uq + 1) * qc)
                    R_sb = rsp.tile([64, qc], F32, tag="rsb", bufs=2, name="R_sb")
                    for c in range(qc // nch):
                        csl = slice(c * nch, (c + 1) * nch)
                        R = x_pool.tile([64, nch], F32, tag="x", bufs=2, name="Rp")
                        MM(R[:], ones_row[0:1, 0:64], urp[0:1, csl], start=True, stop=True)
                        nc.vector.tensor_copy(R_sb[:, csl], R[:])
                    nc.vector.tensor_mul(
                        AN[up][uj * 64 : (uj + 1) * 64, uqsl], uau_sb[0:64, :], R_sb[:]
                    )

                # ---------- fill schedules ----------
                def v6_items():
                    return [
                        (lambda c=c, hf=hf: v6_half(c, hf))
                        for c in range(tokt)
                        for hf in range(2)
                    ]

                def qk_items(p_, m, cs):
                    return [
                        (lambda c=c, hf=hf: qk_half(p_, m, c, hf))
                        for c in cs
                        for hf in range(2)
                    ]

                def op_items(cs):
                    return [
                        (lambda c=c, hf=hf: oproj_half(c, hf))
                        for c in cs
                        for hf in range(2)
                    ]

                v6h = v6_items()
                fills = {i: [] for i in range(2 * HL)}
                # unit 0 absorbs the rest of KT pair0 (chunk c ready before
                # S(4c) consumes it) and all V~ builds (V6[c] ready before the
                # trailing PV(c))
                fills[0] = (
                    qk_items(0, 1, [1]) + v6h[0:4]
                    + qk_items(0, 1, [2]) + v6h[4:12]
                    + qk_items(0, 1, [3]) + v6h[12:32]
                )
                fills[1] = qk_items(1, 1, range(ncc)) + qk_items(1, 0, [0, 1])
                fills[2] = qk_items(2, 1, range(ncc))              # KT pair2
                fills[3] = qk_items(2, 0, [0, 1]) + qk_items(1, 0, [2, 3])
                fills[4] = qk_items(2, 0, [2, 3]) + qk_items(0, 0, [2, 3])
                fills[7] = op_items([0, 1])
                fills[8] = op_items([2, 3])
                fills[9] = op_items([4, 5])
                fills[10] = op_items([6, 7])
                fills[11] = [
                    (lambda c=c, hf=hf: oproj_a_half(c, hf))
                    for c in range(tokt // 2, tokt)
                    for hf in range(2)
                ]

                # ---- projection preamble: KT pair0 chunk0, QT pair0 q0-half ----
                qk_half(0, 1, 0, 0)
                qk_half(0, 1, 0, 1)
                for c in (0, 1):
                    qk_half(0, 0, c, 0)
                    qk_half(0, 0, c, 1)

                # ---- attention units, q-major ----
                units = [
                    (q, p_, j)
                    for q in range(nqc)
                    for p_ in range(NPAIR)
                    for j in range(2)
                ]
                pending = None
                unit_no = 0
                for ui, (q, p_, j) in enumerate(units):
                    h = 2 * p_ + j
                    fl = fills[ui]
                    au = u_pool.tile([65, qc], F32, tag="au", bufs=1, name="au")

                    def emit_pv(okb, opt):
                        for c in range(qc // nch):
                            MM(
                                au[:, c * nch : (c + 1) * nch],
                                V6[okb][:, h * (HD + 1) : (h + 1) * (HD + 1)],
                                opt[:, c * nch : (c + 1) * nch],
                                start=(okb == 0),
                                stop=(okb == tokt - 1),
                            )

                    # PV emission trails exp by L steps: the PE queue is
                    # in-order, so a trailing PV gives the PE ready work while
                    # exp(kb) is still running, and in unit 0 it gives the V~
                    # fills time to land before their PV consumes them.
                    L = 6 if ui == 0 else 2
                    pvq = []
                    for kb in range(tokt):
                        if kb == 10 and pending is not None:
                            finish_unit(pending)
                            pending = None
                        st = s_pool.tile([128, qc], F32, tag="st", bufs=2, name="st")
                        for c in range(qc // nch):
                            MM(
                                st[:, c * nch : (c + 1) * nch],
                                KT[h][:, kb * 128 : (kb + 1) * 128],
                                QT[p_][:, q * qc + c * nch : q * qc + (c + 1) * nch],
                                start=True,
                                stop=True,
                            )
                        pt = ptp.tile([128, qc], BF16, tag="pt", bufs=8, name="pt")
                        nc.scalar.activation(
                            pt[:], st[:], mybir.ActivationFunctionType.Exp, scale=0.125
                        )
                        pvq.append((kb, pt))
                        # fills sit in the PE's exp-wait window (before PV);
                        # pace them so the per-kb PE surplus stays under the
                        # ACT slack (one ~0.6us item per 2 kb when possible)
                        if ui == 0:
                            npop = 2 if kb < 8 else 3
                        elif len(fl) > 16 - kb:
                            npop = 1
                        elif fl and kb % 2 == 1:
                            npop = 1
                        else:
                            npop = 0
                        for _ in range(npop):
                            if fl:
                                fl.pop(0)()
                        if len(pvq) > L:
                            emit_pv(*pvq.pop(0))
                    while fl:
                        fl.pop(0)()
                    for okb, opt in pvq:
                        emit_pv(okb, opt)
                    # unit end: evict au, launch the reciprocal chain
                    au_sb = au_sbs[unit_no % 2]
                    nc.vector.tensor_copy(au_sb[:], au[:])
                    rp_t = r_pads[unit_no % 2]
                    den8 = den8s[unit_no % 2]
                    rec8 = rec8s[unit_no % 2]
                    nc.sync.dma_start(out=den8[:], in_=au_sb[64:65, :])
                    nc.vector.reciprocal(rec8[:], den8[:])
                    nc.sync.dma_start(out=rp_t[0:1, :], in_=rec8[:])
                    if pending is not None:
                        finish_unit(pending)
                    pending = (p_, j, q, au_sb, rp_t)
                    unit_no += 1
                if pending is not None:
                    finish_unit(pending)

                # ---- tail: pass B of the last query chunk's out-projection
                # (pair 2 only; pass A ran as fills in the last unit). The
                # adds alternate DVE/GpSimd so neither engine serializes it.
                for c in range(tokt // 2, tokt):
                    for hf in range(2):
                        oproj_b_half(c, hf, nc.vector)
        lp.__exit__(None, None, None)

    return nc


def shard_inputs(x, w_qkv, b_qkv, w_out, b_out, t=T):
    """Build the 8 per-core input maps. Core = (batch, head-group)."""
    in_maps = []
    for core in range(NCORES):
        b, g = divmod(core, 2)
        hbase = HL * g * HD          # first qk column of this group (384*g)
        # q cols then k cols, pair-interleaved: M-tile 2p = q of heads (2p,2p+1),
        # M-tile 2p+1 = k of the same heads.
        wqk = np.empty((D, 2 * HL * HD), dtype=np.float32)
        bqk = np.empty((2 * HL * HD,), dtype=np.float32)
        for p in range(NPAIR):
            qcols = slice(0 * D + hbase + p * 128, 0 * D + hbase + (p + 1) * 128)
            kcols = slice(1 * D + hbase + p * 128, 1 * D + hbase + (p + 1) * 128)
            wqk[:, (2 * p) * 128 : (2 * p + 1) * 128] = w_qkv[:, qcols]
            wqk[:, (2 * p + 1) * 128 : (2 * p + 2) * 128] = w_qkv[:, kcols]
            bqk[(2 * p) * 128 : (2 * p + 1) * 128] = b_qkv[qcols]
            bqk[(2 * p + 1) * 128 : (2 * p + 2) * 128] = b_qkv[kcols]
        nmt = 2 * HL * HD // 128
        bqk_col = np.ascontiguousarray(bqk.reshape(nmt, 128).T)  # [128, nmt]

        vcols = slice(2 * D + hbase, 2 * D + hbase + HL * HD)
        wv = np.empty((D + 1, HL * HD), dtype=np.float32)
        wv[:D] = w_qkv[:, vcols]
        wv[D] = b_qkv[vcols]

        import ml_dtypes

        bf = ml_dtypes.bfloat16
        wo = np.ascontiguousarray(w_out[hbase : hbase + HL * HD, :]).astype(bf)

        in_maps.append(
            {
                "xt": np.ascontiguousarray(x[b, :t].T).astype(bf),
                "wqk": wqk.astype(bf),
                "bqk": bqk_col,
                "wv": wv.astype(bf),
                "wo": wo,
            }
        )
    return in_maps


def kernel(x, w_qkv, b_qkv, w_out, b_out):
    x = np.asarray(x, dtype=np.float32)
    w_qkv = np.asarray(w_qkv, dtype=np.float32)
    b_qkv = np.asarray(b_qkv, dtype=np.float32)
    w_out = np.asarray(w_out, dtype=np.float32)
    b_out = np.asarray(b_out, dtype=np.float32)

    nc = build_nc()
    _split_multi_waits(nc)
    in_maps = shard_inputs(x, w_qkv, b_qkv, w_out, b_out)
    res = run_bass_kernel_spmd(nc, in_maps, list(range(NCORES)))
    parts = [np.asarray(res.results[i]["out"]) for i in range(NCORES)]
    out = np.stack([parts[2 * b] + parts[2 * b + 1] for b in range(B)], axis=0)
    out += b_out[None, None, :]
    return out.astype(np.float32)


# revision 45
# speedup vs baseline: 1.1713x; 1.0020x over previous
"""Trainium2 Bass kernel: multi-head attention (B=4, T=2048, D=768, H=12).

Sharding: 8 cores = 4 batches x 2 head-groups (6 heads each).
Each core computes QKV projection (its heads), attention, and a partial
output projection (contraction over its 384 of 768 w_out rows).
Host unshard: out[b] = partial[2b] + partial[2b+1] (bias folded on host).

v2: single software-pipelined schedule. The attention phase is bound by
the Scalar (ACT) engine's exp throughput (~1.1us per [128,1024] tile,
192 tiles = ~212us). All other PE work (QKV projection, V~ build,
output projection) is micro-chunked (<=4 matmuls per item) and
interleaved into the per-kb exp-wait windows of the attention loop, so
the PE's idle slack under the ACT-bound steady state absorbs it.

Per-core dataflow:
  - Q^T/K^T in transposed pair-packed [128, T] bf16 tiles; K^T per-head
    zero-padded so S^T matmuls run with a full 128-partition contraction
  - V~ natural layout with a ones column per head ([tok, 65]) so P@V also
    produces softmax denominators (row 64 of au)
  - S^T = K^T.T @ Q^T -> exp via ScalarE (fused PSUM eviction, scale=1/8,
    no max-subtraction; scores bounded ~[-2.5, 2.8]) -> au^T = V~.T @ P^T
  - unit end: au PSUM -> SBUF copy (frees the single au PSUM slot), the
    denominator row is DMA-reshaped [1,1024]->[8,128] so the reciprocal
    runs on 8 DVE lanes, DMA'd back to a [1,1024] row, broadcast to 64
    partitions with a contraction-1 matmul against an all-ones row, and
    multiplied into the out-proj lhsT layout (deferred by one unit so the
    PE never waits on the reciprocal chain)
  - out-proj from attnN^T pair tiles; b_out added on host during unshard
  - matmuls in float32r (1 cycle/row at N>=256); f32r constant fills go
    through f32 twins + DVE copies (memset cannot write f32r)

Schedule: units ordered q-major (all 6 heads of query-chunk 0, then
chunk 1). Unit 0 absorbs the V~ builds just-in-time (V6[kb] completes
right before its own PV(kb)); units 1-4 absorb the remaining projection
pairs; units 7-10 absorb the out-projection of query-chunk 0; the
out-projection of chunk 1 is the tail.

This walrus build encodes at most one sync wait per instruction; Tile
emits several. _split_multi_waits() rewrites the final module, hoisting
extra waits onto same-engine nops inserted before the instruction.
"""

import numpy as np

import concourse.bass as bass
import concourse.mybir as mybir
from concourse.tile import TileContext
from concourse.bass_utils import run_bass_kernel_spmd

# problem constants (fixed by the graded nn.Module)
B, T, D = 4, 2048, 768
H, HD = 12, 64
NCORES = 8
HL = H // 2            # heads per core (2 head-groups)
NPAIR = HL // 2        # head pairs per core

F32 = mybir.dt.float32
F32R = mybir.dt.float32r
BF16 = mybir.dt.bfloat16


def _split_multi_waits(nc):
    """Walrus here encodes only one sync wait per instruction. Move extra
    waits onto same-engine nops placed immediately before the instruction."""
    n = 0
    for f in nc.m.functions:
        for bb in f.blocks:
            new = []
            for inst in bb.instructions:
                si = inst.sync_info
                if si is not None and si.on_wait and len(si.on_wait) > 1:
                    extra = list(si.on_wait[:-1])
                    keep = si.on_wait[-1]
                    del si.on_wait[:]
                    si.on_wait.append(keep)
                    for w in extra:
                        nop = mybir.InstNoOp(name=f"I-wsplit-{n}", ins=[], outs=[])
                        n += 1
                        nop.engine = inst.engine
                        nop.sync_info = mybir.SyncInfo(on_wait=[w], on_update=[])
                        new.append(nop)
                new.append(inst)
            bb.instructions[:] = new
    return n


def build_nc(t=T, qc=1024, nch=512):
    """Build the SPMD per-core program. qc = attention query chunk,
    nch = matmul moving-dim chunk."""
    tokt = t // 128            # token tiles
    nqc = t // qc              # query chunks
    dk = D // 128              # contraction tiles over D
    ncc = t // nch             # projection moving chunks per M row
    nmt = 2 * HL * HD // 128   # QK projection M-tiles (6)

    nc = bass.Bass("TRN2", target_bir_lowering=False, debug=False)

    xt_d = nc.dram_tensor("xt", [D, t], BF16, kind="ExternalInput")
    wqk_d = nc.dram_tensor("wqk", [D, 2 * HL * HD], BF16, kind="ExternalInput")
    bqk_d = nc.dram_tensor("bqk", [128, nmt], F32, kind="ExternalInput")
    wv_d = nc.dram_tensor("wv", [D + 1, HL * HD], BF16, kind="ExternalInput")
    wo_d = nc.dram_tensor("wo", [HL * HD, D], BF16, kind="ExternalInput")
    out_d = nc.dram_tensor("out", [t, D], BF16, kind="ExternalOutput")

    def MM(out, lhsT, rhs, start, stop):
        nc.tensor.matmul(out, lhsT, rhs, start=start, stop=stop)

    with TileContext(nc) as tc:
        lp = nc.allow_low_precision(reason="float32r matmul operand production")
        lp.__enter__()
        with tc.tile_pool(name="persist", bufs=1) as pp:
            ones_row = pp.tile([1, 128], F32R, name="ones_row")
            ones_bf = pp.tile([1, 128], BF16, name="ones_bf")
            QT = [pp.tile([128, t], BF16, name=f"qt{p}") for p in range(NPAIR)]
            KT = [pp.tile([128, t], BF16, name=f"kt{h}") for h in range(HL)]
            V6 = [pp.tile([128, HL * (HD + 1)], BF16, name=f"v6_{c}") for c in range(tokt)]
            bqk_t = pp.tile([128, nmt], F32, name="bqk_t")
            AN = [pp.tile([128, t], BF16, name=f"an{p}") for p in range(NPAIR)]
            WO = [pp.tile([128, D], BF16, name=f"wop{p}") for p in range(NPAIR)]
            r_pads = [pp.tile([1, qc], F32R, name=f"r_pad{i}") for i in range(2)]
            au_sbs = [pp.tile([65, qc], F32, name=f"au_sb{i}") for i in range(2)]
            den8s = [pp.tile([8, qc // 8], F32, name=f"den8_{i}") for i in range(2)]
            rec8s = [pp.tile([8, qc // 8], F32R, name=f"rec8_{i}") for i in range(2)]
            # pair-0/1 partial sums of the tail out-projection (pass A runs
            # as fills inside the last unit; pass B adds pair 2 at the tail)
            soa = [pp.tile([128, D], BF16, name=f"soa{i}") for i in range(tokt // 2)]
            xt_t = pp.tile([128, dk, t], BF16, name="xt_t")
            wqk_t = pp.tile([128, dk, 2 * HL * HD], BF16, name="wqk_t")
            wv_t = pp.tile([128, dk, HL * HD], BF16, name="wv_t")
            wvb = pp.tile([1, HL * HD], BF16, name="wvb")

            # ---- DMA emission in priority order: the first S matmul needs
            # KT[0] chunk0 + QT[0] cols 0:1024 (xt chunks 0-1 + wqk pair0);
            # V~ tiles need wv; xt chunks 2-3 aren't consumed until kb 8+.
            nc.sync.dma_start(out=bqk_t[:], in_=bqk_d[:, :])

            def dma_wqk(psl):
                nc.sync.dma_start(
                    out=wqk_t[:, :, psl],
                    in_=wqk_d[:, psl].rearrange("(k r) c -> r k c", k=dk),
                )

            def dma_xt(ch):
                csl = slice(ch * nch, (ch + 1) * nch)
                nc.sync.dma_start(
                    out=xt_t[:, :, csl],
                    in_=xt_d[:, csl].rearrange("(k r) c -> r k c", k=dk),
                )

            # KT pair0 (m-tile 1) feeds the very first matmuls: its weight
            # slice goes first, the QT (m-tile 0) slice after xt chunk 0
            dma_wqk(slice(128, 256))
            dma_xt(0)
            dma_wqk(slice(0, 128))
            dma_xt(1)
            nc.sync.dma_start(
                out=wv_t[:], in_=wv_d[0:D, :].rearrange("(k r) c -> r k c", k=dk)
            )
            nc.sync.dma_start(out=wvb[0:1, :], in_=wv_d[D : D + 1, :])
            dma_xt(2)
            dma_xt(3)
            for p_ in (1, 2):
                dma_wqk(slice(p_ * 256, (p_ + 1) * 256))
            for p_ in range(NPAIR):
                nc.sync.dma_start(out=WO[p_][:], in_=wo_d[p_ * 128 : (p_ + 1) * 128, :])

            # ---- constants / padding init ----
            with tc.tile_pool(name="init", bufs=1) as ip:
                ones32 = ip.tile([1, 128], F32, name="ones32")
                nc.vector.memset(ones32[:], 1.0)
                nc.vector.tensor_copy(ones_row[:], ones32[:])
                nc.vector.memset(ones_bf[:], 1.0)
                warm = ip.tile([1, 16], F32, name="warm")
                nc.scalar.activation(
                    warm[:], ones32[0:1, 0:16], mybir.ActivationFunctionType.Exp
                )

                for h in range(HL):
                    if h % 2 == 0:
                        nc.vector.memset(KT[h][64:128, :], 0.0)
                    else:
                        nc.vector.memset(KT[h][0:64, :], 0.0)
                for c in range(tokt):
                    v3i = V6[c][:].rearrange("p (h c) -> p h c", c=HD + 1)
                    nc.vector.memset(v3i[:, :, HD : HD + 1], 1.0)

            with (
                tc.tile_pool(name="ps_s", bufs=2, space="PSUM") as s_pool,
                tc.tile_pool(name="ps_u", bufs=1, space="PSUM") as u_pool,
                tc.tile_pool(name="ps_x", bufs=2, space="PSUM") as x_pool,
                tc.tile_pool(name="sb_pt", bufs=3) as ptp,
                tc.tile_pool(name="sb_r", bufs=2) as rsp,
                tc.tile_pool(name="sb_o", bufs=3) as osp,
            ):
                # ---------- micro-item emitters ----------
                aux_state = {}

                def qk_half(p_, m, c, half):
                    """Half of one QK-projection chunk: 3 of 6 k-matmuls into
                    an aux PSUM slot; second half evicts to QT/KT."""
                    key = ("qk", p_, m, c)
                    gm = 2 * p_ + m
                    csl = slice(c * nch, (c + 1) * nch)
                    if half == 0:
                        ps = x_pool.tile([128, nch], F32, tag="x", bufs=2, name="psqk")
                        aux_state[key] = ps
                        ks = range(0, dk // 2)
                    else:
                        ps = aux_state.pop(key)
                        ks = range(dk // 2, dk)
                    for k in ks:
                        MM(
                            ps[:],
                            wqk_t[:, k, gm * 128 : (gm + 1) * 128],
                            xt_t[:, k, csl],
                            start=(k == 0),
                            stop=(k == dk - 1),
                        )
                    if half == 1:
                        if m == 0:
                            nc.vector.tensor_scalar_add(
                                QT[p_][:, csl], ps[:], bqk_t[:, gm : gm + 1]
                            )
                        else:
                            h0, h1 = 2 * p_, 2 * p_ + 1
                            nc.vector.tensor_scalar_add(
                                KT[h0][0:64, csl], ps[0:64, :], bqk_t[0:64, gm : gm + 1]
                            )
                            nc.vector.tensor_scalar_add(
                                KT[h1][64:128, csl], ps[64:128, :], bqk_t[64:128, gm : gm + 1]
                            )

                def v6_half(c, half):
                    """Half of one V~ tile build: k-matmuls into aux PSUM;
                    second half adds bias (contraction-1 matmul) and scatters
                    into V6[c] with the per-head ones column."""
                    key = ("v6", c)
                    tsl = slice(c * 128, (c + 1) * 128)
                    if half == 0:
                        psv = x_pool.tile(
                            [128, HL * HD], F32, tag="x", bufs=2, name="psv"
                        )
                        aux_state[key] = psv
                        for k in range(0, dk // 2):
                            MM(psv[:], xt_t[:, k, tsl], wv_t[:, k, :], start=(k == 0), stop=False)
                    else:
                        psv = aux_state.pop(key)
                        for k in range(dk // 2, dk):
                            MM(psv[:], xt_t[:, k, tsl], wv_t[:, k, :], start=False, stop=False)
                        MM(psv[:], ones_bf[0:1, 0:128], wvb[0:1, :], start=False, stop=True)
                        v3 = V6[c][:].rearrange("p (h c) -> p h c", c=HD + 1)
                        nc.vector.tensor_copy(
                            v3[:, :, 0:HD],
                            psv[:].rearrange("p (h c) -> p h c", c=HD),
                        )

                def oproj_half(c, half):
                    """Half of one out-projection token tile: 3 pair-matmuls
                    over one 384-column slice of D, evicted into a staging
                    SBUF tile; second half DMAs the tile out."""
                    key = ("op", c)
                    tsl = slice(c * 128, (c + 1) * 128)
                    nsl = slice(half * (D // 2), (half + 1) * (D // 2))
                    ps = x_pool.tile([128, D // 2], F32, tag="x", bufs=2, name="pso")
                    if half == 0:
                        so = osp.tile([128, D], BF16, tag="so", bufs=3, name="so")
                        aux_state[key] = so
                    else:
                        so = aux_state.pop(key)
                    for p_ in range(NPAIR):
                        MM(
                            ps[:],
                            AN[p_][:, tsl],
                            WO[p_][:, nsl],
                            start=(p_ == 0),
                            stop=(p_ == NPAIR - 1),
                        )
                    nc.vector.tensor_copy(so[:, nsl], ps[:])
                    if half == 1:
                        nc.sync.dma_start(out=out_d[tsl, :], in_=so[:])

                def oproj_a_half(c, half):
                    """Tail out-proj pass A: pairs 0+1 only, into a persistent
                    SBUF accumulator (pair 2's AN isn't normalized yet)."""
                    tsl = slice(c * 128, (c + 1) * 128)
                    nsl = slice(half * (D // 2), (half + 1) * (D // 2))
                    ps = x_pool.tile([128, D // 2], F32, tag="x", bufs=2, name="psa")
                    for p_ in (0, 1):
                        MM(ps[:], AN[p_][:, tsl], WO[p_][:, nsl], start=(p_ == 0), stop=(p_ == 1))
                    nc.vector.tensor_copy(soa[c - tokt // 2][:, nsl], ps[:])

                def oproj_b_half(c, half, eng):
                    """Tail out-proj pass B: pair 2 matmul + add of pass A."""
                    key = ("opb", c)
                    tsl = slice(c * 128, (c + 1) * 128)
                    nsl = slice(half * (D // 2), (half + 1) * (D // 2))
                    ps = x_pool.tile([128, D // 2], F32, tag="x", bufs=2, name="psb")
                    if half == 0:
                        so = osp.tile([128, D], BF16, tag="so", bufs=3, name="so")
                        aux_state[key] = so
                    else:
                        so = aux_state.pop(key)
                    MM(ps[:], AN[2][:, tsl], WO[2][:, nsl], start=True, stop=True)
                    eng.tensor_tensor(
                        so[:, nsl], ps[:], soa[c - tokt // 2][:, nsl],
                        op=mybir.AluOpType.add,
                    )
                    if half == 1:
                        nc.sync.dma_start(out=out_d[tsl, :], in_=so[:])

                def finish_unit(u):
                    """Deferred normalize: broadcast the reciprocal row to 64
                    partitions (contraction-1 matmuls), multiply into AN."""
                    up, uj, uq, uau_sb, urp = u
                    uqsl = slice(uq * qc, (uq + 1) * qc)
                    R_sb = rsp.tile([64, qc], F32, tag="rsb", bufs=2, name="R_sb")
                    for c in range(qc // nch):
                        csl = slice(c * nch, (c + 1) * nch)
                        R = x_pool.tile([64, nch], F32, tag="x", bufs=2, name="Rp")
                        MM(R[:], ones_row[0:1, 0:64], urp[0:1, csl], start=True, stop=True)
                        nc.vector.tensor_copy(R_sb[:, csl], R[:])
                    nc.vector.tensor_mul(
                        AN[up][uj * 64 : (uj + 1) * 64, uqsl], uau_sb[0:64, :], R_sb[:]
                    )

                # ---------- fill schedules ----------
                def v6_items():
                    return [
                        (lambda c=c, hf=hf: v6_half(c, hf))
                        for c in range(tokt)
                        for hf in range(2)
                    ]

                def qk_items(p_, m, cs):
                    return [
                        (lambda c=c, hf=hf: qk_half(p_, m, c, hf))
                        for c in cs
                        for hf in range(2)
                    ]

                def op_items(cs):
                    return [
                        (lambda c=c, hf=hf: oproj_half(c, hf))
                        for c in cs
                        for hf in range(2)
                    ]

                v6h = v6_items()
                fills = {i: [] for i in range(2 * HL)}
                # unit 0 absorbs the rest of KT pair0 (chunk c ready before
                # S(4c) consumes it) and all V~ builds (V6[c] ready before the
                # trailing PV(c))
                fills[0] = (
                    qk_items(0, 1, [1]) + v6h[0:4]
                    + qk_items(0, 1, [2]) + v6h[4:12]
                    + qk_items(0, 1, [3]) + v6h[12:32]
                )
                fills[1] = qk_items(1, 1, range(ncc)) + qk_items(1, 0, [0, 1])
                fills[2] = qk_items(2, 1, range(ncc))              # KT pair2
                fills[3] = qk_items(2, 0, [0, 1]) + qk_items(1, 0, [2, 3])
                fills[4] = qk_items(2, 0, [2, 3]) + qk_items(0, 0, [2, 3])
                fills[7] = op_items([0, 1])
                fills[8] = op_items([2, 3])
                fills[9] = op_items([4, 5])
                fills[10] = op_items([6, 7])
                fills[11] = [
                    (lambda c=c, hf=hf: oproj_a_half(c, hf))
                    for c in range(tokt // 2, tokt)
                    for hf in range(2)
                ]

                # ---- projection preamble: KT pair0 chunk0, QT pair0 q0-half ----
                qk_half(0, 1, 0, 0)
                qk_half(0, 1, 0, 1)
                for c in (0, 1):
                    qk_half(0, 0, c, 0)
                    qk_half(0, 0, c, 1)

                # ---- attention units, q-major ----
                units = [
                    (q, p_, j)
                    for q in range(nqc)
                    for p_ in range(NPAIR)
                    for j in range(2)
                ]
                pending = None
                unit_no = 0
                for ui, (q, p_, j) in enumerate(units):
                    h = 2 * p_ + j
                    fl = fills[ui]
                    au = u_pool.tile([65, qc], F32, tag="au", bufs=1, name="au")

                    def emit_pv(okb, opt):
                        for c in range(qc // nch):
                            MM(
                                au[:, c * nch : (c + 1) * nch],
                                V6[okb][:, h * (HD + 1) : (h + 1) * (HD + 1)],
                                opt[:, c * nch : (c + 1) * nch],
                                start=(okb == 0),
                                stop=(okb == tokt - 1),
                            )

                    # PV emission trails exp by L steps: the PE queue is
                    # in-order, so a trailing PV gives the PE ready work while
                    # exp(kb) is still running, and in unit 0 it gives the V~
                    # fills time to land before their PV consumes them.
                    L = 6 if ui == 0 else 2
                    pvq = []
                    for kb in range(tokt):
                        if kb == 10 and pending is not None:
                            finish_unit(pending)
                            pending = None
                        st = s_pool.tile([128, qc], F32, tag="st", bufs=2, name="st")
                        for c in range(qc // nch):
                            MM(
                                st[:, c * nch : (c + 1) * nch],
                                KT[h][:, kb * 128 : (kb + 1) * 128],
                                QT[p_][:, q * qc + c * nch : q * qc + (c + 1) * nch],
                                start=True,
                                stop=True,
                            )
                        pt = ptp.tile([128, qc], BF16, tag="pt", bufs=8, name="pt")
                        nc.scalar.activation(
                            pt[:], st[:], mybir.ActivationFunctionType.Exp, scale=0.125
                        )
                        pvq.append((kb, pt))
                        # fills sit in the PE's exp-wait window (before PV);
                        # pace them so the per-kb PE surplus stays under the
                        # ACT slack (one ~0.6us item per 2 kb when possible)
                        if ui == 0:
                            npop = 2 if kb < 8 else 3
                        elif len(fl) > 16 - kb:
                            npop = 1
                        elif fl and kb % 2 == 1:
                            npop = 1
                        else:
                            npop = 0
                        for _ in range(npop):
                            if fl:
                                fl.pop(0)()
                        if len(pvq) > L:
                            emit_pv(*pvq.pop(0))
                    while fl:
                        fl.pop(0)()
                    for okb, opt in pvq:
                        emit_pv(okb, opt)
                    # unit end: evict au, launch the reciprocal chain
                    au_sb = au_sbs[unit_no % 2]
                    nc.vector.tensor_copy(au_sb[:], au[:])
                    rp_t = r_pads[unit_no % 2]
                    den8 = den8s[unit_no % 2]
                    rec8 = rec8s[unit_no % 2]
                    nc.sync.dma_start(out=den8[:], in_=au_sb[64:65, :])
                    nc.vector.reciprocal(rec8[:], den8[:])
                    nc.sync.dma_start(out=rp_t[0:1, :], in_=rec8[:])
                    if pending is not None:
                        finish_unit(pending)
                    pending = (p_, j, q, au_sb, rp_t)
                    unit_no += 1
                if pending is not None:
                    finish_unit(pending)

                # ---- tail: pass B of the last query chunk's out-projection
                # (pair 2 only; pass A ran as fills in the last unit). The
                # adds alternate DVE/GpSimd so neither engine serializes it.
                for c in range(tokt // 2, tokt):
                    for hf in range(2):
                        oproj_b_half(c, hf, nc.vector)
        lp.__exit__(None, None, None)

    return nc


def shard_inputs(x, w_qkv, b_qkv, w_out, b_out, t=T):
    """Build the 8 per-core input maps. Core = (batch, head-group)."""
    in_maps = []
    for core in range(NCORES):
        b, g = divmod(core, 2)
        hbase = HL * g * HD          # first qk column of this group (384*g)
        # q cols then k cols, pair-interleaved: M-tile 2p = q of heads (2p,2p+1),
        # M-tile 2p+1 = k of the same heads.
        wqk = np.empty((D, 2 * HL * HD), dtype=np.float32)
        bqk = np.empty((2 * HL * HD,), dtype=np.float32)
        for p in range(NPAIR):
            qcols = slice(0 * D + hbase + p * 128, 0 * D + hbase + (p + 1) * 128)
            kcols = slice(1 * D + hbase + p * 128, 1 * D + hbase + (p + 1) * 128)
            wqk[:, (2 * p) * 128 : (2 * p + 1) * 128] = w_qkv[:, qcols]
            wqk[:, (2 * p + 1) * 128 : (2 * p + 2) * 128] = w_qkv[:, kcols]
            bqk[(2 * p) * 128 : (2 * p + 1) * 128] = b_qkv[qcols]
            bqk[(2 * p + 1) * 128 : (2 * p + 2) * 128] = b_qkv[kcols]
        nmt = 2 * HL * HD // 128
        bqk_col = np.ascontiguousarray(bqk.reshape(nmt, 128).T)  # [128, nmt]

        vcols = slice(2 * D + hbase, 2 * D + hbase + HL * HD)
        wv = np.empty((D + 1, HL * HD), dtype=np.float32)
        wv[:D] = w_qkv[:, vcols]
        wv[D] = b_qkv[vcols]

        import ml_dtypes

        bf = ml_dtypes.bfloat16
        wo = np.ascontiguousarray(w_out[hbase : hbase + HL * HD, :]).astype(bf)

        in_maps.append(
            {
                "xt": np.ascontiguousarray(x[b, :t].T).astype(bf),
                "wqk": wqk.astype(bf),
                "bqk": bqk_col,
                "wv": wv.astype(bf),
                "wo": wo,
            }
        )
    return in_maps


def kernel(x, w_qkv, b_qkv, w_out, b_out):
    x = np.asarray(x, dtype=np.float32)
    w_qkv = np.asarray(w_qkv, dtype=np.float32)
    b_qkv = np.asarray(b_qkv, dtype=np.float32)
    w_out = np.asarray(w_out, dtype=np.float32)
    b_out = np.asarray(b_out, dtype=np.float32)

    nc = build_nc()
    _split_multi_waits(nc)
    in_maps = shard_inputs(x, w_qkv, b_qkv, w_out, b_out)
    res = run_bass_kernel_spmd(nc, in_maps, list(range(NCORES)))
    parts = [np.asarray(res.results[i]["out"]).astype(np.float32) for i in range(NCORES)]
    out = np.stack([parts[2 * b] + parts[2 * b + 1] for b in range(B)], axis=0)
    out += b_out[None, None, :]
    return out.astype(np.float32)


# revision 46
# speedup vs baseline: 1.1759x; 1.0039x over previous
"""Trainium2 Bass kernel: multi-head attention (B=4, T=2048, D=768, H=12).

Sharding: 8 cores = 4 batches x 2 head-groups (6 heads each).
Each core computes QKV projection (its heads), attention, and a partial
output projection (contraction over its 384 of 768 w_out rows).
Host unshard: out[b] = partial[2b] + partial[2b+1] (bias folded on host).

v2: single software-pipelined schedule. The attention phase is bound by
the Scalar (ACT) engine's exp throughput (~1.1us per [128,1024] tile,
192 tiles = ~212us). All other PE work (QKV projection, V~ build,
output projection) is micro-chunked (<=4 matmuls per item) and
interleaved into the per-kb exp-wait windows of the attention loop, so
the PE's idle slack under the ACT-bound steady state absorbs it.

Per-core dataflow:
  - Q^T/K^T in transposed pair-packed [128, T] bf16 tiles; K^T per-head
    zero-padded so S^T matmuls run with a full 128-partition contraction
  - V~ natural layout with a ones column per head ([tok, 65]) so P@V also
    produces softmax denominators (row 64 of au)
  - S^T = K^T.T @ Q^T -> exp via ScalarE (fused PSUM eviction, scale=1/8,
    no max-subtraction; scores bounded ~[-2.5, 2.8]) -> au^T = V~.T @ P^T
  - unit end: au PSUM -> SBUF copy (frees the single au PSUM slot), the
    denominator row is DMA-reshaped [1,1024]->[8,128] so the reciprocal
    runs on 8 DVE lanes, DMA'd back to a [1,1024] row, broadcast to 64
    partitions with a contraction-1 matmul against an all-ones row, and
    multiplied into the out-proj lhsT layout (deferred by one unit so the
    PE never waits on the reciprocal chain)
  - out-proj from attnN^T pair tiles; b_out added on host during unshard
  - matmuls in float32r (1 cycle/row at N>=256); f32r constant fills go
    through f32 twins + DVE copies (memset cannot write f32r)

Schedule: units ordered q-major (all 6 heads of query-chunk 0, then
chunk 1). Unit 0 absorbs the V~ builds just-in-time (V6[kb] completes
right before its own PV(kb)); units 1-4 absorb the remaining projection
pairs; units 7-10 absorb the out-projection of query-chunk 0; the
out-projection of chunk 1 is the tail.

This walrus build encodes at most one sync wait per instruction; Tile
emits several. _split_multi_waits() rewrites the final module, hoisting
extra waits onto same-engine nops inserted before the instruction.
"""

import numpy as np

import concourse.bass as bass
import concourse.mybir as mybir
from concourse.tile import TileContext
from concourse.bass_utils import run_bass_kernel_spmd

# problem constants (fixed by the graded nn.Module)
B, T, D = 4, 2048, 768
H, HD = 12, 64
NCORES = 8
HL = H // 2            # heads per core (2 head-groups)
NPAIR = HL // 2        # head pairs per core

F32 = mybir.dt.float32
F32R = mybir.dt.float32r
BF16 = mybir.dt.bfloat16


def _split_multi_waits(nc):
    """Walrus here encodes only one sync wait per instruction. Move extra
    waits onto same-engine nops placed immediately before the instruction."""
    n = 0
    for f in nc.m.functions:
        for bb in f.blocks:
            new = []
            for inst in bb.instructions:
                si = inst.sync_info
                if si is not None and si.on_wait and len(si.on_wait) > 1:
                    extra = list(si.on_wait[:-1])
                    keep = si.on_wait[-1]
                    del si.on_wait[:]
                    si.on_wait.append(keep)
                    for w in extra:
                        nop = mybir.InstNoOp(name=f"I-wsplit-{n}", ins=[], outs=[])
                        n += 1
                        nop.engine = inst.engine
                        nop.sync_info = mybir.SyncInfo(on_wait=[w], on_update=[])
                        new.append(nop)
                new.append(inst)
            bb.instructions[:] = new
    return n


def build_nc(t=T, qc=1024, nch=512):
    """Build the SPMD per-core program. qc = attention query chunk,
    nch = matmul moving-dim chunk."""
    tokt = t // 128            # token tiles
    nqc = t // qc              # query chunks
    dk = D // 128              # contraction tiles over D
    ncc = t // nch             # projection moving chunks per M row
    nmt = 2 * HL * HD // 128   # QK projection M-tiles (6)

    nc = bass.Bass("TRN2", target_bir_lowering=False, debug=False)

    xt_d = nc.dram_tensor("xt", [D, t], BF16, kind="ExternalInput")
    wqk_d = nc.dram_tensor("wqk", [D, 2 * HL * HD], BF16, kind="ExternalInput")
    bqk_d = nc.dram_tensor("bqk", [128, nmt], F32, kind="ExternalInput")
    wv_d = nc.dram_tensor("wv", [D + 1, HL * HD], BF16, kind="ExternalInput")
    wo_d = nc.dram_tensor("wo", [HL * HD, D], BF16, kind="ExternalInput")
    out_d = nc.dram_tensor("out", [t, D], BF16, kind="ExternalOutput")

    def MM(out, lhsT, rhs, start, stop):
        nc.tensor.matmul(out, lhsT, rhs, start=start, stop=stop)

    with TileContext(nc) as tc:
        lp = nc.allow_low_precision(reason="float32r matmul operand production")
        lp.__enter__()
        with tc.tile_pool(name="persist", bufs=1) as pp:
            ones_row = pp.tile([1, 128], F32R, name="ones_row")
            ones_bf = pp.tile([1, 128], BF16, name="ones_bf")
            QT = [pp.tile([128, t], BF16, name=f"qt{p}") for p in range(NPAIR)]
            KT = [pp.tile([128, t], BF16, name=f"kt{h}") for h in range(HL)]
            V6 = [pp.tile([128, HL * (HD + 1)], BF16, name=f"v6_{c}") for c in range(tokt)]
            bqk_t = pp.tile([128, nmt], F32, name="bqk_t")
            AN = [pp.tile([128, t], BF16, name=f"an{p}") for p in range(NPAIR)]
            WO = [pp.tile([128, D], BF16, name=f"wop{p}") for p in range(NPAIR)]
            r_pads = [pp.tile([1, qc], F32R, name=f"r_pad{i}") for i in range(2)]
            au_sbs = [pp.tile([65, qc], F32, name=f"au_sb{i}") for i in range(2)]
            den8s = [pp.tile([8, qc // 8], F32, name=f"den8_{i}") for i in range(2)]
            rec8s = [pp.tile([8, qc // 8], F32R, name=f"rec8_{i}") for i in range(2)]
            # pair-0/1 partial sums of the tail out-projection (pass A runs
            # as fills inside the last unit; pass B adds pair 2 at the tail)
            soa = [pp.tile([128, D], BF16, name=f"soa{i}") for i in range(tokt // 2)]
            xt_t = pp.tile([128, dk, t], BF16, name="xt_t")
            wqk_t = pp.tile([128, dk, 2 * HL * HD], BF16, name="wqk_t")
            wv_t = pp.tile([128, dk, HL * HD], BF16, name="wv_t")
            wvb = pp.tile([1, HL * HD], BF16, name="wvb")

            # ---- DMA emission in priority order: the first S matmul needs
            # KT[0] chunk0 + QT[0] cols 0:1024 (xt chunks 0-1 + wqk pair0);
            # V~ tiles need wv; xt chunks 2-3 aren't consumed until kb 8+.
            nc.sync.dma_start(out=bqk_t[:], in_=bqk_d[:, :])

            def dma_wqk(psl):
                nc.sync.dma_start(
                    out=wqk_t[:, :, psl],
                    in_=wqk_d[:, psl].rearrange("(k r) c -> r k c", k=dk),
                )

            def dma_xt(ch):
                csl = slice(ch * nch, (ch + 1) * nch)
                nc.sync.dma_start(
                    out=xt_t[:, :, csl],
                    in_=xt_d[:, csl].rearrange("(k r) c -> r k c", k=dk),
                )

            # KT pair0 (m-tile 1) feeds the very first matmuls: its weight
            # slice goes first, the QT (m-tile 0) slice after xt chunk 0
            dma_wqk(slice(128, 256))
            dma_xt(0)
            dma_wqk(slice(0, 128))
            dma_xt(1)
            nc.sync.dma_start(
                out=wv_t[:], in_=wv_d[0:D, :].rearrange("(k r) c -> r k c", k=dk)
            )
            nc.sync.dma_start(out=wvb[0:1, :], in_=wv_d[D : D + 1, :])
            dma_xt(2)
            dma_xt(3)
            for p_ in (1, 2):
                dma_wqk(slice(p_ * 256, (p_ + 1) * 256))
            for p_ in range(NPAIR):
                nc.sync.dma_start(out=WO[p_][:], in_=wo_d[p_ * 128 : (p_ + 1) * 128, :])

            # ---- constants / padding init ----
            with tc.tile_pool(name="init", bufs=1) as ip:
                ones32 = ip.tile([1, 128], F32, name="ones32")
                nc.vector.memset(ones32[:], 1.0)
                nc.vector.tensor_copy(ones_row[:], ones32[:])
                nc.vector.memset(ones_bf[:], 1.0)
                warm = ip.tile([1, 16], F32, name="warm")
                nc.scalar.activation(
                    warm[:], ones32[0:1, 0:16], mybir.ActivationFunctionType.Exp
                )

                for h in range(HL):
                    if h % 2 == 0:
                        nc.vector.memset(KT[h][64:128, :], 0.0)
                    else:
                        nc.vector.memset(KT[h][0:64, :], 0.0)
                for c in range(tokt):
                    v3i = V6[c][:].rearrange("p (h c) -> p h c", c=HD + 1)
                    nc.vector.memset(v3i[:, :, HD : HD + 1], 1.0)

            with (
                tc.tile_pool(name="ps_s", bufs=2, space="PSUM") as s_pool,
                tc.tile_pool(name="ps_u", bufs=1, space="PSUM") as u_pool,
                tc.tile_pool(name="ps_x", bufs=2, space="PSUM") as x_pool,
                tc.tile_pool(name="sb_pt", bufs=3) as ptp,
                tc.tile_pool(name="sb_r", bufs=2) as rsp,
                tc.tile_pool(name="sb_o", bufs=3) as osp,
            ):
                # ---------- micro-item emitters ----------
                aux_state = {}

                def qk_half(p_, m, c, half):
                    """Half of one QK-projection chunk: 3 of 6 k-matmuls into
                    an aux PSUM slot; second half evicts to QT/KT."""
                    key = ("qk", p_, m, c)
                    gm = 2 * p_ + m
                    csl = slice(c * nch, (c + 1) * nch)
                    if half == 0:
                        ps = x_pool.tile([128, nch], F32, tag="x", bufs=2, name="psqk")
                        aux_state[key] = ps
                        ks = range(0, dk // 2)
                    else:
                        ps = aux_state.pop(key)
                        ks = range(dk // 2, dk)
                    for k in ks:
                        MM(
                            ps[:],
                            wqk_t[:, k, gm * 128 : (gm + 1) * 128],
                            xt_t[:, k, csl],
                            start=(k == 0),
                            stop=(k == dk - 1),
                        )
                    if half == 1:
                        if m == 0:
                            nc.vector.tensor_scalar_add(
                                QT[p_][:, csl], ps[:], bqk_t[:, gm : gm + 1]
                            )
                        else:
                            h0, h1 = 2 * p_, 2 * p_ + 1
                            nc.vector.tensor_scalar_add(
                                KT[h0][0:64, csl], ps[0:64, :], bqk_t[0:64, gm : gm + 1]
                            )
                            nc.vector.tensor_scalar_add(
                                KT[h1][64:128, csl], ps[64:128, :], bqk_t[64:128, gm : gm + 1]
                            )

                def v6_half(c, half):
                    """Half of one V~ tile build: k-matmuls into aux PSUM;
                    second half adds bias (contraction-1 matmul) and scatters
                    into V6[c] with the per-head ones column."""
                    key = ("v6", c)
                    tsl = slice(c * 128, (c + 1) * 128)
                    if half == 0:
                        psv = x_pool.tile(
                            [128, HL * HD], F32, tag="x", bufs=2, name="psv"
                        )
                        aux_state[key] = psv
                        for k in range(0, dk // 2):
                            MM(psv[:], xt_t[:, k, tsl], wv_t[:, k, :], start=(k == 0), stop=False)
                    else:
                        psv = aux_state.pop(key)
                        for k in range(dk // 2, dk):
                            MM(psv[:], xt_t[:, k, tsl], wv_t[:, k, :], start=False, stop=False)
                        MM(psv[:], ones_bf[0:1, 0:128], wvb[0:1, :], start=False, stop=True)
                        v3 = V6[c][:].rearrange("p (h c) -> p h c", c=HD + 1)
                        nc.vector.tensor_copy(
                            v3[:, :, 0:HD],
                            psv[:].rearrange("p (h c) -> p h c", c=HD),
                        )

                def oproj_half(c, half):
                    """Half of one out-projection token tile: 3 pair-matmuls
                    over one 384-column slice of D, evicted into a staging
                    SBUF tile; second half DMAs the tile out."""
                    key = ("op", c)
                    tsl = slice(c * 128, (c + 1) * 128)
                    nsl = slice(half * (D // 2), (half + 1) * (D // 2))
                    ps = x_pool.tile([128, D // 2], F32, tag="x", bufs=2, name="pso")
                    if half == 0:
                        so = osp.tile([128, D], BF16, tag="so", bufs=3, name="so")
                        aux_state[key] = so
                    else:
                        so = aux_state.pop(key)
                    for p_ in range(NPAIR):
                        MM(
                            ps[:],
                            AN[p_][:, tsl],
                            WO[p_][:, nsl],
                            start=(p_ == 0),
                            stop=(p_ == NPAIR - 1),
                        )
                    nc.vector.tensor_copy(so[:, nsl], ps[:])
                    if half == 1:
                        nc.sync.dma_start(out=out_d[tsl, :], in_=so[:])

                def oproj_a_half(c, half):
                    """Tail out-proj pass A: pairs 0+1 only, into a persistent
                    SBUF accumulator (pair 2's AN isn't normalized yet)."""
                    tsl = slice(c * 128, (c + 1) * 128)
                    nsl = slice(half * (D // 2), (half + 1) * (D // 2))
                    ps = x_pool.tile([128, D // 2], F32, tag="x", bufs=2, name="psa")
                    for p_ in (0, 1):
                        MM(ps[:], AN[p_][:, tsl], WO[p_][:, nsl], start=(p_ == 0), stop=(p_ == 1))
                    nc.vector.tensor_copy(soa[c - tokt // 2][:, nsl], ps[:])

                def oproj_b_half(c, half, eng):
                    """Tail out-proj pass B: pair 2 matmul + add of pass A."""
                    key = ("opb", c)
                    tsl = slice(c * 128, (c + 1) * 128)
                    nsl = slice(half * (D // 2), (half + 1) * (D // 2))
                    ps = x_pool.tile([128, D // 2], F32, tag="x", bufs=2, name="psb")
                    if half == 0:
                        so = osp.tile([128, D], BF16, tag="so", bufs=3, name="so")
                        aux_state[key] = so
                    else:
                        so = aux_state.pop(key)
                    MM(ps[:], AN[2][:, tsl], WO[2][:, nsl], start=True, stop=True)
                    eng.tensor_tensor(
                        so[:, nsl], ps[:], soa[c - tokt // 2][:, nsl],
                        op=mybir.AluOpType.add,
                    )
                    if half == 1:
                        nc.sync.dma_start(out=out_d[tsl, :], in_=so[:])

                def finish_unit(u):
                    """Deferred normalize: broadcast the reciprocal row to 64
                    partitions (contraction-1 matmuls), multiply into AN."""
                    up, uj, uq, uau_sb, urp = u
                    uqsl = slice(uq * qc, (uq + 1) * qc)
                    R_sb = rsp.tile([64, qc], F32, tag="rsb", bufs=2, name="R_sb")
                    for c in range(qc // nch):
                        csl = slice(c * nch, (c + 1) * nch)
                        R = x_pool.tile([64, nch], F32, tag="x", bufs=2, name="Rp")
                        MM(R[:], ones_row[0:1, 0:64], urp[0:1, csl], start=True, stop=True)
                        nc.vector.tensor_copy(R_sb[:, csl], R[:])
                    nc.vector.tensor_mul(
                        AN[up][uj * 64 : (uj + 1) * 64, uqsl], uau_sb[0:64, :], R_sb[:]
                    )

                # ---------- fill schedules ----------
                def v6_items():
                    return [
                        (lambda c=c, hf=hf: v6_half(c, hf))
                        for c in range(tokt)
                        for hf in range(2)
                    ]

                def qk_items(p_, m, cs):
                    return [
                        (lambda c=c, hf=hf: qk_half(p_, m, c, hf))
                        for c in cs
                        for hf in range(2)
                    ]

                def op_items(cs):
                    return [
                        (lambda c=c, hf=hf: oproj_half(c, hf))
                        for c in cs
                        for hf in range(2)
                    ]

                v6h = v6_items()
                fills = {i: [] for i in range(2 * HL)}
                # unit 0 absorbs the rest of KT pair0 (chunk c ready before
                # S(4c) consumes it) and all V~ builds (V6[c] ready before the
                # trailing PV(c))
                fills[0] = (
                    qk_items(0, 1, [1]) + v6h[0:4]
                    + qk_items(0, 1, [2]) + v6h[4:12]
                    + qk_items(0, 1, [3]) + v6h[12:32]
                )
                fills[1] = qk_items(1, 1, range(ncc)) + qk_items(1, 0, [0, 1])
                fills[2] = qk_items(2, 1, range(ncc))              # KT pair2
                fills[3] = qk_items(2, 0, [0, 1]) + qk_items(1, 0, [2, 3])
                fills[4] = qk_items(2, 0, [2, 3]) + qk_items(0, 0, [2, 3])
                fills[7] = op_items([0, 1])
                fills[8] = op_items([2, 3])
                fills[9] = op_items([4, 5])
                fills[10] = op_items([6, 7])
                fills[11] = [
                    (lambda c=c, hf=hf: oproj_a_half(c, hf))
                    for c in range(tokt // 2, tokt)
                    for hf in range(2)
                ]

                # ---- PE p-state warmup: the PE clock ramps with sustained
                # use (0.65 -> 1.2 -> 2.4 GHz over ~3us). These dummy
                # matmuls run during the input-DMA wait so the projection
                # preamble starts at full clock instead of cold.
                wps = x_pool.tile([128, 128], F32, tag="x", bufs=2, name="wps")
                for wi in range(16):
                    MM(
                        wps[:],
                        ones_row[0:1, 0:128],
                        ones_row[0:1, 0:128],
                        start=(wi == 0),
                        stop=(wi == 15),
                    )

                # ---- projection preamble: KT pair0 chunk0, QT pair0 q0-half ----
                qk_half(0, 1, 0, 0)
                qk_half(0, 1, 0, 1)
                for c in (0, 1):
                    qk_half(0, 0, c, 0)
                    qk_half(0, 0, c, 1)

                # ---- attention units, q-major ----
                units = [
                    (q, p_, j)
                    for q in range(nqc)
                    for p_ in range(NPAIR)
                    for j in range(2)
                ]
                pending = None
                unit_no = 0
                for ui, (q, p_, j) in enumerate(units):
                    h = 2 * p_ + j
                    fl = fills[ui]
                    au = u_pool.tile([65, qc], F32, tag="au", bufs=1, name="au")

                    def emit_pv(okb, opt):
                        for c in range(qc // nch):
                            MM(
                                au[:, c * nch : (c + 1) * nch],
                                V6[okb][:, h * (HD + 1) : (h + 1) * (HD + 1)],
                                opt[:, c * nch : (c + 1) * nch],
                                start=(okb == 0),
                                stop=(okb == tokt - 1),
                            )

                    # PV emission trails exp by L steps: the PE queue is
                    # in-order, so a trailing PV gives the PE ready work while
                    # exp(kb) is still running, and in unit 0 it gives the V~
                    # fills time to land before their PV consumes them.
                    L = 6 if ui == 0 else 2
                    pvq = []
                    for kb in range(tokt):
                        if kb == 10 and pending is not None:
                            finish_unit(pending)
                            pending = None
                        st = s_pool.tile([128, qc], F32, tag="st", bufs=2, name="st")
                        for c in range(qc // nch):
                            MM(
                                st[:, c * nch : (c + 1) * nch],
                                KT[h][:, kb * 128 : (kb + 1) * 128],
                                QT[p_][:, q * qc + c * nch : q * qc + (c + 1) * nch],
                                start=True,
                                stop=True,
                            )
                        pt = ptp.tile([128, qc], BF16, tag="pt", bufs=8, name="pt")
                        nc.scalar.activation(
                            pt[:], st[:], mybir.ActivationFunctionType.Exp, scale=0.125
                        )
                        pvq.append((kb, pt))
                        # fills sit in the PE's exp-wait window (before PV);
                        # pace them so the per-kb PE surplus stays under the
                        # ACT slack (one ~0.6us item per 2 kb when possible)
                        if ui == 0:
                            npop = 2 if kb < 8 else 3
                        elif len(fl) > 16 - kb:
                            npop = 1
                        elif fl and kb % 2 == 1:
                            npop = 1
                        else:
                            npop = 0
                        for _ in range(npop):
                            if fl:
                                fl.pop(0)()
                        if len(pvq) > L:
                            emit_pv(*pvq.pop(0))
                    while fl:
                        fl.pop(0)()
                    for okb, opt in pvq:
                        emit_pv(okb, opt)
                    # unit end: evict au, launch the reciprocal chain
                    au_sb = au_sbs[unit_no % 2]
                    nc.vector.tensor_copy(au_sb[:], au[:])
                    rp_t = r_pads[unit_no % 2]
                    den8 = den8s[unit_no % 2]
                    rec8 = rec8s[unit_no % 2]
                    nc.sync.dma_start(out=den8[:], in_=au_sb[64:65, :])
                    nc.vector.reciprocal(rec8[:], den8[:])
                    nc.sync.dma_start(out=rp_t[0:1, :], in_=rec8[:])
                    if pending is not None:
                        finish_unit(pending)
                    pending = (p_, j, q, au_sb, rp_t)
                    unit_no += 1
                if pending is not None:
                    finish_unit(pending)

                # ---- tail: pass B of the last query chunk's out-projection
                # (pair 2 only; pass A ran as fills in the last unit). The
                # adds alternate DVE/GpSimd so neither engine serializes it.
                for c in range(tokt // 2, tokt):
                    for hf in range(2):
                        oproj_b_half(c, hf, nc.vector)
        lp.__exit__(None, None, None)

    return nc


def shard_inputs(x, w_qkv, b_qkv, w_out, b_out, t=T):
    """Build the 8 per-core input maps. Core = (batch, head-group)."""
    in_maps = []
    for core in range(NCORES):
        b, g = divmod(core, 2)
        hbase = HL * g * HD          # first qk column of this group (384*g)
        # q cols then k cols, pair-interleaved: M-tile 2p = q of heads (2p,2p+1),
        # M-tile 2p+1 = k of the same heads.
        wqk = np.empty((D, 2 * HL * HD), dtype=np.float32)
        bqk = np.empty((2 * HL * HD,), dtype=np.float32)
        for p in range(NPAIR):
            qcols = slice(0 * D + hbase + p * 128, 0 * D + hbase + (p + 1) * 128)
            kcols = slice(1 * D + hbase + p * 128, 1 * D + hbase + (p + 1) * 128)
            wqk[:, (2 * p) * 128 : (2 * p + 1) * 128] = w_qkv[:, qcols]
            wqk[:, (2 * p + 1) * 128 : (2 * p + 2) * 128] = w_qkv[:, kcols]
            bqk[(2 * p) * 128 : (2 * p + 1) * 128] = b_qkv[qcols]
            bqk[(2 * p + 1) * 128 : (2 * p + 2) * 128] = b_qkv[kcols]
        nmt = 2 * HL * HD // 128
        bqk_col = np.ascontiguousarray(bqk.reshape(nmt, 128).T)  # [128, nmt]

        vcols = slice(2 * D + hbase, 2 * D + hbase + HL * HD)
        wv = np.empty((D + 1, HL * HD), dtype=np.float32)
        wv[:D] = w_qkv[:, vcols]
        wv[D] = b_qkv[vcols]

        import ml_dtypes

        bf = ml_dtypes.bfloat16
        wo = np.ascontiguousarray(w_out[hbase : hbase + HL * HD, :]).astype(bf)

        in_maps.append(
            {
                "xt": np.ascontiguousarray(x[b, :t].T).astype(bf),
                "wqk": wqk.astype(bf),
                "bqk": bqk_col,
                "wv": wv.astype(bf),
                "wo": wo,
            }
        )
    return in_maps


def kernel(x, w_qkv, b_qkv, w_out, b_out):
    x = np.asarray(x, dtype=np.float32)
    w_qkv = np.asarray(w_qkv, dtype=np.float32)
    b_qkv = np.asarray(b_qkv, dtype=np.float32)
    w_out = np.asarray(w_out, dtype=np.float32)
    b_out = np.asarray(b_out, dtype=np.float32)

    nc = build_nc()
    _split_multi_waits(nc)
    in_maps = shard_inputs(x, w_qkv, b_qkv, w_out, b_out)
    res = run_bass_kernel_spmd(nc, in_maps, list(range(NCORES)))
    parts = [np.asarray(res.results[i]["out"]).astype(np.float32) for i in range(NCORES)]
    out = np.stack([parts[2 * b] + parts[2 * b + 1] for b in range(B)], axis=0)
    out += b_out[None, None, :]
    return out.astype(np.float32)
